# revision 22
# baseline (speedup 1.0000x reference)
"""Sliding-window GQA attention (Gemma-style) on 8 Trainium2 NeuronCores.

Sharding: data-parallel over tokens. B=2, T=2048 -> 4096 tokens -> 512
queries per core (core c = 4*b + j handles batch b, queries [512j, 512j+512)).
Each core recomputes k/v for its fixed local window of 1536 tokens
[qs-1024, qs+512) (zero-padded on the left at sequence start), so all 8 cores
run one identical NEFF; per-core differences live entirely in the input data
(sliced x, masks with validity baked in, RoPE tables).

On-chip dataflow (per core):
  phase 1: q/k/v projections with W stationary and x^T moving -> q^T/k^T/v^T
           [H=128 partitions, tokens]; fused RMSNorm (ones-matmul column
           sums + gpsimd partition-broadcast of 1/std) and RoPE (head-dim
           permuted on host so the rotate-half is a quadrant-local
           stream_shuffle); v^T transposed back to [s, h] via PE transposes.
  phase 2: logits^T = k^T.T @ q^T per (head, q-tile) -> tanh softcap + exp on
           ACT -> mask multiply (host-built masks) -> denominators via
           ones-matmul over s-partitions -> P^T @ ... PV accumulation ->
           encT scaled by 1/den on evacuation.
  phase 3: output projection accumulating over heads, DMA straight from PSUM.
"""

import numpy as np
import ml_dtypes

import concourse.bass as bass
import concourse.mybir as mybir
import concourse.tile as tile
from concourse import library_config
from concourse.masks import make_identity
from concourse.bass_utils import run_bass_kernel_spmd

AF = mybir.ActivationFunctionType
ALU = mybir.AluOpType
F32 = mybir.dt.float32
BF16 = mybir.dt.bfloat16
FP8 = mybir.dt.float8e4

B, T, D = 2, 2048, 2048
N, K, H = 16, 8, 128
G = N // K
SOFT_CAP = 50.0
WINDOW = 1024
SCALE = H ** -0.5
ROPE_BASE = 10000.0
EPS = 1e-6

TQ = 512            # queries per core
TKV = 1536          # kv window per core
VST = 129           # per-s-tile width in vsb: 128 v cols + ones column
NQT = TQ // 128     # 4 q-tiles
NST = TKV // 128    # 12 s-tiles
ND = D // 128       # 16 d-tiles
NWIN = 9            # s-tiles in a q-tile's window
NCORES = 8

# quadrant-local half swap for stream_shuffle (32-partition groups)
SWAP16 = list(range(16, 32)) + list(range(16))


def _rope_perm():
    """orig[p] = original head-dim index stored at partition p; freq[p];
    sign[p] for the sin table."""
    orig = np.zeros(128, np.int64)
    freq = np.zeros(128, np.int64)
    sign = np.zeros(128, np.float32)
    for p in range(128):
        qd, o = divmod(p, 32)
        if o < 16:
            orig[p] = 16 * qd + o
            freq[p] = 16 * qd + o
            sign[p] = -1.0
        else:
            orig[p] = 64 + 16 * qd + (o - 16)
            freq[p] = 16 * qd + (o - 16)
            sign[p] = 1.0
    return orig, freq, sign


_ORIG, _FREQ, _SIGN = _rope_perm()

_module_cache = {}

# Instruction types lowered to CTRL encodings: the walrus build in this
# container supports only ONE sync-wait on them ("Too many sync wait
# commands" / "ISA wrong length" in codegen otherwise).
_CTRL_TYPES = ("InstDrain", "InstNoOp", "InstISA", "InstEventSemaphore")


def _split_ctrl_multiwaits(nc, maxw=1):
    """Move excess sem-waits off CTRL-type instructions onto preceding
    same-engine NoOps (same engine queue => identical ordering semantics)."""
    import concourse.mybir as mybir
    for f in nc.m.functions:
        for blk in f.blocks:
            insts = blk.instructions
            out = []
            changed = False
            for inst in insts:
                si = inst.sync_info
                if (si is not None and si.on_wait
                        and len(si.on_wait) > maxw):
                    waits = list(si.on_wait)
                    extra, keep = waits[:-maxw], waits[-maxw:]
                    for k, w in enumerate(extra):
                        nop = mybir.InstNoOp(name=f"{inst.name}-ws{k}",
                                             ins=[], outs=[])
                        nop.engine = inst.engine
                        nop.sync_info = mybir.SyncInfo(on_wait=[w],
                                                       on_update=[])
                        out.append(nop)
                    si.on_wait = keep
                    changed = True
                out.append(inst)
            if changed:
                insts[:] = out


def _build_module(split=True):
    nc = bass.Bass("TRN2", target_bir_lowering=False, debug=False)

    xt_d = nc.dram_tensor("xt", (D, TKV), BF16, kind="ExternalInput").ap()
    wq_d = nc.dram_tensor("wq", (N, D, H), BF16, kind="ExternalInput").ap()
    wk_d = nc.dram_tensor("wk", (K, D, H), BF16, kind="ExternalInput").ap()
    wv_d = nc.dram_tensor("wv", (K, D, H), BF16, kind="ExternalInput").ap()
    wo_d = nc.dram_tensor("wo", (N, H, D), BF16, kind="ExternalInput").ap()
    gq_d = nc.dram_tensor("gq", (H, 1), F32, kind="ExternalInput").ap()
    gk_d = nc.dram_tensor("gk", (H, 1), F32, kind="ExternalInput").ap()
    ck_d = nc.dram_tensor("ck", (H, TKV), BF16, kind="ExternalInput").ap()
    sk_d = nc.dram_tensor("sk", (H, TKV), BF16, kind="ExternalInput").ap()
    mk_d = nc.dram_tensor("mk", (NQT, 128, NWIN * 128), FP8,
                          kind="ExternalInput").ap()
    idb_d = nc.dram_tensor("idb", (128, 128), BF16, kind="ExternalInput").ap()
    out_d = nc.dram_tensor("out", (TQ, D), F32, kind="ExternalOutput").ap()

    with tile.TileContext(nc) as tc:
        with tc.tile_pool(name="const", bufs=1) as cst, \
             tc.tile_pool(name="acc", bufs=1) as acc, \
             tc.tile_pool(name="wst", bufs=2) as wst, \
             tc.tile_pool(name="wost", bufs=3) as wost, \
             tc.tile_pool(name="scr", bufs=2) as scr, \
             tc.tile_pool(name="pp", bufs=3) as pp, \
             tc.tile_pool(name="psA", bufs=4, space="PSUM") as psA, \
             tc.tile_pool(name="psB", bufs=4, space="PSUM") as psB:

            # ---- constants / preloads ----
            xts = cst.tile([128, ND * TKV], BF16, tag="xts")
            xt_r = xt_d.rearrange("(d p) t -> d p t", p=128)
            for d in range(ND):
                nc.sync.dma_start(xts[:, d * TKV:(d + 1) * TKV], xt_r[d])
            gq_t = cst.tile([H, 1], F32, tag="gq")
            nc.sync.dma_start(gq_t[:], gq_d[:])
            gk_t = cst.tile([H, 1], F32, tag="gk")
            nc.sync.dma_start(gk_t[:], gk_d[:])
            ck_t = cst.tile([H, TKV], BF16, tag="ck")
            nc.sync.dma_start(ck_t[:], ck_d[:])
            sk_t = cst.tile([H, TKV], BF16, tag="sk")
            nc.sync.dma_start(sk_t[:], sk_d[:])
            mk_t = cst.tile([128, NQT * NWIN * 128], FP8, tag="mk")
            nc.sync.dma_start(
                mk_t[:].rearrange("p (q m) -> p q m", q=NQT),
                mk_d.rearrange("q p m -> p q m"))
            idb_t = cst.tile([128, 128], BF16, tag="idb")
            nc.sync.dma_start(idb_t[:], idb_d[:])
            ones_bf = cst.tile([128, 1], BF16, tag="ones")
            nc.vector.memset(ones_bf[:], 1.0)
            on1 = cst.tile([1, 128], F32, tag="on1")
            nc.vector.memset(on1[:], 1.0)
            id1 = cst.tile([1, 1], F32, tag="id1")
            nc.vector.memset(id1[:], 1.0)
            eps_t = cst.tile([1, 1], F32, tag="eps")
            nc.vector.memset(eps_t[:], EPS)


            # ---- big accumulators ----
            qTn = acc.tile([128, N * TQ], BF16, tag="qTn")
            kTn = acc.tile([128, K * TKV], BF16, tag="kTn")
            vsb = acc.tile([128, K * NST * VST], BF16, tag="vsb")
            nc.vector.memset(vsb[:], 1.0)
            encT = acc.tile([128, N * NQT * 128], BF16, tag="encT")
            # per-s-tile 1/(50*std_k) columns, [128 s, K*NST]
            rkc = acc.tile([128, K * NST], F32, tag="rkc")

            def rope(src_bf, c0, out_slice):
                rot = scr.tile([128, 512], BF16, tag="rot")
                nc.vector.stream_shuffle(rot[:], src_bf[:], SWAP16)
                t1 = scr.tile([128, 512], BF16, tag="t1")
                nc.vector.tensor_mul(t1[:], src_bf[:], ck_t[:, c0:c0 + 512])
                t2 = scr.tile([128, 512], BF16, tag="t2")
                nc.vector.tensor_mul(t2[:], rot[:], sk_t[:, c0:c0 + 512])
                nc.vector.tensor_add(out_slice, t1[:], t2[:])

            def sumsq_rows(ps):
                """ps [128,512] f32 psum -> std [1,512] f32 = sqrt(ms+eps)."""
                sq = scr.tile([128, 512], BF16, tag="sq")
                nc.scalar.activation(sq[:], ps[:], AF.Square)
                ssp = psA.tile([1, 512], F32, tag="big")
                nc.tensor.matmul(ssp[:], ones_bf[:], sq[:],
                                 start=True, stop=True)
                std = scr.tile([1, 512], F32, tag="row")
                nc.scalar.activation(std[:], ssp[:], AF.Sqrt,
                                     scale=1.0 / H, bias=eps_t[:])
                return std

            # ---- phase 1: q heads (norm fully applied on q) ----
            q_s1 = q_s2 = None
            for n in range(N + 2):
                nstate = None
                if n < N:
                    w_t = wst.tile([128, ND * H], BF16, tag="w")
                    nc.sync.dma_start(
                        w_t[:].rearrange("p (d h) -> p d h", d=ND),
                        wq_d[n].rearrange("(d p) h -> p d h", p=128))
                    ps = psA.tile([128, 512], F32, tag="big")
                    for d in range(ND):
                        nc.tensor.matmul(
                            ps[:], w_t[:, d * H:(d + 1) * H],
                            xts[:, d * TKV + 1024:d * TKV + 1536],
                            start=(d == 0), stop=(d == ND - 1))
                    praw = scr.tile([128, 512], F32, tag="praw")
                    nc.vector.tensor_copy(praw[:], ps[:])
                    sq = scr.tile([128, 512], BF16, tag="sq")
                    nc.scalar.activation(sq[:], ps[:], AF.Square)
                    nstate = (n, praw, sq)
                if q_s1 is not None:
                    n1, praw1, sq1 = q_s1
                    ssp = psA.tile([1, 512], F32, tag="big")
                    nc.tensor.matmul(ssp[:], ones_bf[:], sq1[:],
                                     start=True, stop=True)
                    std = scr.tile([1, 512], F32, tag="row")
                    nc.scalar.activation(std[:], ssp[:], AF.Sqrt,
                                         scale=1.0 / H, bias=eps_t[:])
                    lnt = scr.tile([1, 512], F32, tag="row")
                    nc.scalar.activation(lnt[:], std[:], AF.Ln)
                    rst = scr.tile([1, 512], F32, tag="row")
                    nc.scalar.activation(rst[:], lnt[:], AF.Exp, scale=-1.0)
                    q_s1 = (n1, praw1, rst)
                if q_s2 is not None:
                    n2, praw2, rst2 = q_s2
                    rbp = psA.tile([128, 512], F32, tag="big")
                    nc.tensor.matmul(rbp[:], on1[:], rst2[:],
                                     start=True, stop=True)
                    qn = scr.tile([128, 512], BF16, tag="qn")
                    nc.vector.scalar_tensor_tensor(
                        qn[:], praw2[:], gq_t[:], rbp[:],
                        op0=ALU.mult, op1=ALU.mult)
                    rope(qn, 1024, qTn[:, n2 * TQ:(n2 + 1) * TQ])
                q_s2 = q_s1
                q_s1 = nstate
            k_s1 = k_s2 = None
            rkps = {}
            nchunks = K * 3
            for ci in range(nchunks + 2):
                nstate = None
                if ci < nchunks:
                    kh, c = divmod(ci, 3)
                    if c == 0:
                        w_t = wst.tile([128, ND * H], BF16, tag="w")
                        nc.sync.dma_start(
                            w_t[:].rearrange("p (d h) -> p d h", d=ND),
                            wk_d[kh].rearrange("(d p) h -> p d h", p=128))
                        rkps[kh] = psB.tile([128, NST], F32, tag="sm",
                                            name=f"rkp_{kh}")
                    ps = psA.tile([128, 512], F32, tag="big")
                    for d in range(ND):
                        nc.tensor.matmul(
                            ps[:], w_t[:, d * H:(d + 1) * H],
                            xts[:, d * TKV + c * 512:d * TKV + (c + 1) * 512],
                            start=(d == 0), stop=(d == ND - 1))
                    kn = scr.tile([128, 512], BF16, tag="kn")
                    nc.vector.tensor_scalar_mul(kn[:], ps[:], gk_t[:])
                    sq = scr.tile([128, 512], BF16, tag="sq")
                    nc.scalar.activation(sq[:], ps[:], AF.Square)
                    nstate = (kh, c, kn, sq)
                if k_s1 is not None:
                    kh1, c1, kn1, sq1 = k_s1
                    ssp = psA.tile([1, 512], F32, tag="big")
                    nc.tensor.matmul(ssp[:], ones_bf[:], sq1[:],
                                     start=True, stop=True)
                    std = scr.tile([1, 512], F32, tag="row")
                    nc.scalar.activation(std[:], ssp[:], AF.Sqrt,
                                         scale=1.0 / H, bias=eps_t[:])
                    k_s1 = (kh1, c1, kn1, std)
                if k_s2 is not None:
                    kh2, c2, kn2, std2 = k_s2
                    rkp2 = rkps[kh2]
                    for t4 in range(4):
                        st = c2 * 4 + t4
                        nc.tensor.matmul(
                            rkp2[:, st:st + 1],
                            std2[:, t4 * 128:(t4 + 1) * 128], id1[:],
                            is_transpose=True, start=True, stop=True)
                    rope(kn2, c2 * 512,
                         kTn[:, kh2 * TKV + c2 * 512:kh2 * TKV + (c2 + 1) * 512])
                    if c2 == 2:
                        rkraw = scr.tile([128, NST], F32, tag="rkraw")
                        nc.scalar.copy(rkraw[:], rkp2[:])
                        nc.vector.reciprocal(
                            rkc[:, kh2 * NST:(kh2 + 1) * NST], rkraw[:])
                        del rkps[kh2]
                k_s2 = k_s1
                k_s1 = nstate
            vstate = None
            for ci in range(nchunks + 1):
                nstate = None
                if ci < nchunks:
                    kh, c = divmod(ci, 3)
                    if c == 0:
                        w_t = wst.tile([128, ND * H], BF16, tag="w")
                        nc.sync.dma_start(
                            w_t[:].rearrange("p (d h) -> p d h", d=ND),
                            wv_d[kh].rearrange("(d p) h -> p d h", p=128))
                    ps = psA.tile([128, 512], F32, tag="big")
                    for d in range(ND):
                        nc.tensor.matmul(
                            ps[:], w_t[:, d * H:(d + 1) * H],
                            xts[:, d * TKV + c * 512:d * TKV + (c + 1) * 512],
                            start=(d == 0), stop=(d == ND - 1))
                    vt_sb = scr.tile([128, 512], BF16, tag="vt")
                    nc.vector.tensor_copy(vt_sb[:], ps[:])
                    nstate = (kh, c, vt_sb)
                if vstate is not None:
                    kh0, c0, vt0 = vstate
                    for t4 in range(4):
                        st = c0 * 4 + t4
                        tps = psB.tile([128, 128], BF16, tag="sm")
                        nc.tensor.matmul(
                            tps[:], vt0[:, t4 * 128:(t4 + 1) * 128],
                            idb_t[:], is_transpose=True,
                            start=True, stop=True)
                        off = (kh0 * NST + st) * VST
                        nc.scalar.copy(vsb[:, off:off + 128], tps[:])
                vstate = nstate

            # ---- phase 2: attention (PV staggered one iteration behind QK) ----
            a_s1 = a_s2 = None
            iters = [(n, qi) for n in range(N) for qi in range(NQT)]
            for it in range(len(iters) + 2):
                nstate = None
                if it < len(iters):
                    n, qi = iters[it]
                    kh = n // G
                    probs = pp.tile([128, NWIN * 128], BF16, tag="probs")
                    for c in range(3):
                        nr = 4 if c < 2 else 1
                        if c < 2:
                            lg = psA.tile([128, 512], F32, tag="big")
                        else:
                            lg = psB.tile([128, 128], F32, tag="sm")
                        for rr in range(nr):
                            r = c * 4 + rr
                            st = qi + r
                            nc.tensor.matmul(
                                lg[:, rr * 128:(rr + 1) * 128],
                                kTn[:, kh * TKV + st * 128:kh * TKV + (st + 1) * 128],
                                qTn[:, n * TQ + qi * 128:n * TQ + (qi + 1) * 128],
                                start=True, stop=True)
                        rk_sl = rkc[:, kh * NST + qi + c * 4:
                                    kh * NST + qi + c * 4 + nr]
                        rk_b = bass.AP(rk_sl.tensor, rk_sl.offset,
                                       list(rk_sl.ap) + [[0, 128]])
                        ttA = scr.tile([128, 512], F32, tag="ttA")
                        nc.vector.tensor_tensor(
                            ttA[:, :nr * 128].rearrange(
                                "p (r t) -> p r t", r=nr),
                            lg[:].rearrange("p (r t) -> p r t", r=nr),
                            rk_b, op=ALU.mult)
                        ttB = scr.tile([128, 512], F32, tag="ttB")
                        nc.scalar.activation(ttB[:, :nr * 128],
                                             ttA[:, :nr * 128],
                                             AF.Tanh, scale=1.0 / SOFT_CAP)
                        ee = scr.tile([128, 512], BF16, tag="ee")
                        nc.scalar.activation(ee[:, :nr * 128],
                                             ttB[:, :nr * 128],
                                             AF.Exp, scale=SOFT_CAP)
                        nc.vector.tensor_mul(
                            probs[:, c * 512:c * 512 + nr * 128],
                            ee[:, :nr * 128],
                            mk_t[:, (qi * NWIN + c * 4) * 128:
                                 (qi * NWIN + c * 4 + nr) * 128])
                    nstate = (n, qi, probs)
                if a_s2 is not None:
                    n0, qi0, probs0 = a_s2
                    kh0 = n0 // G
                    ev = psB.tile([128, VST + 3], F32, tag="sm")
                    for r in range(NWIN):
                        st = qi0 + r
                        off = (kh0 * NST + st) * VST
                        nc.tensor.matmul(
                            ev[:, 0:VST],
                            probs0[:, r * 128:(r + 1) * 128],
                            vsb[:, off:off + VST],
                            start=(r == 0), stop=(r == NWIN - 1))
                    den = scr.tile([128, 1], F32, tag="den")
                    nc.vector.tensor_copy(den[:], ev[:, 128:129])
                    rden = scr.tile([128, 1], F32, tag="rden")
                    nc.vector.reciprocal(rden[:], den[:])
                    enc_sb = scr.tile([128, H], BF16, tag="encsb")
                    nc.vector.tensor_scalar_mul(enc_sb[:], ev[:, 0:H], rden[:])
                    etp = psB.tile([128, 128], BF16, tag="sm")
                    nc.tensor.matmul(etp[:], enc_sb[:], idb_t[:],
                                     is_transpose=True, start=True, stop=True)
                    nc.vector.tensor_copy(
                        encT[:, (n0 * NQT + qi0) * 128:(n0 * NQT + qi0 + 1) * 128],
                        etp[:])
                a_s2 = a_s1
                a_s1 = nstate

            # ---- phase 3: output projection ----
            for dc in range(4):
                ops = [psA.tile([128, 512], F32, tag="big", name=f"op_{dc}_{qi}")
                       for qi in range(NQT)]
                for n in range(N):
                    wo_sl = wost.tile([128, 512], BF16, tag="wo")
                    nc.sync.dma_start(wo_sl[:],
                                      wo_d[n][:, dc * 512:(dc + 1) * 512])
                    for qi in range(NQT):
                        nc.tensor.matmul(
                            ops[qi][:],
                            encT[:, (n * NQT + qi) * 128:(n * NQT + qi + 1) * 128],
                            wo_sl[:], start=(n == 0), stop=(n == N - 1))
                for qi in range(NQT):
                    osb = scr.tile([128, 512], F32, tag="osb")
                    nc.vector.tensor_copy(osb[:], ops[qi][:])
                    nc.sync.dma_start(
                        out_d[qi * 128:(qi + 1) * 128, dc * 512:(dc + 1) * 512],
                        osb[:])

    if split:
        _split_ctrl_multiwaits(nc)
    return nc


def _prep_inputs(x, q_w, kv_w, o_w, qnorm_scale, knorm_scale, segment_pos,
                 attn_mask):
    """Host-side shard + layout prep. Returns list of 8 input dicts."""
    bf = ml_dtypes.bfloat16
    f8 = ml_dtypes.float8_e4m3
    x = np.asarray(x, np.float32)
    q_w = np.asarray(q_w, np.float32)
    kv_w = np.asarray(kv_w, np.float32)
    o_w = np.asarray(o_w, np.float32)
    qnorm_scale = np.asarray(qnorm_scale, np.float32)
    knorm_scale = np.asarray(knorm_scale, np.float32)
    segment_pos = np.asarray(segment_pos, np.int64)
    attn_mask = np.asarray(attn_mask, bool)

    # shared (same array object across cores -> no copy)
    wq = np.ascontiguousarray(q_w[:, :, _ORIG]).astype(bf)
    wk = np.ascontiguousarray(kv_w[0][:, :, _ORIG]).astype(bf)
    wv = kv_w[1].astype(bf)
    wo = o_w.astype(bf)
    gq = ((1.0 + qnorm_scale[_ORIG]) * SCALE).reshape(H, 1).astype(np.float32)
    gk = (1.0 + knorm_scale[_ORIG]).reshape(H, 1).astype(np.float32)
    timescale = ROPE_BASE ** (2.0 * _FREQ.astype(np.float64) / H)  # [128]
    idb = np.eye(128, dtype=bf)

    in_maps = []
    for c in range(NCORES):
        b, j = divmod(c, NQT)
        qs = TQ * j
        kvs = qs - WINDOW

        # x^T for local kv window, zero-padded on the left
        xt = np.zeros((D, TKV), bf)
        lo = max(kvs, 0)
        xt[:, lo - kvs:] = x[b, lo:qs + TQ, :].T.astype(bf)

        # rope tables in permuted row order; positions from segment_pos
        pos = np.zeros(TKV, np.float64)
        pos[lo - kvs:] = segment_pos[b, lo:qs + TQ].astype(np.float64)
        theta = pos[None, :] / timescale[:, None]          # [128, TKV]
        ck = np.cos(theta).astype(bf)
        sk = (np.sin(theta) * _SIGN[:, None]).astype(bf)

        # masks [NQT, 128 s_p, NWIN*128 (r, t)] with validity baked in
        mk = np.zeros((NQT, 128, NWIN * 128), f8)
        seg = segment_pos[b]
        for qi in range(NQT):
            q_glob = qs + qi * 128 + np.arange(128)                  # [t]
            st = qi + np.arange(NWIN)
            k_glob = (kvs + st[:, None] * 128 + np.arange(128)[None, :])
            valid = k_glob >= 0                                       # [r, sp]
            k_safe = np.clip(k_glob, 0, T - 1)
            am = attn_mask[b][np.ix_(q_glob, k_safe.reshape(-1))]     # [t, r*sp]
            pk = seg[k_safe.reshape(-1)]                              # [r*sp]
            pq = seg[q_glob]                                          # [t]
            win = ((pk[None, :] > pq[:, None] - WINDOW)
                   & (pk[None, :] < pq[:, None] + WINDOW))
            m = (am & win & valid.reshape(1, -1)).astype(np.float32)  # [t, r*sp]
            m = m.reshape(128, NWIN, 128).transpose(2, 1, 0)          # [sp, r, t]
            mk[qi] = m.reshape(128, NWIN * 128).astype(f8)

        in_maps.append(dict(
            xt=xt, wq=wq, wk=wk, wv=wv, wo=wo, gq=gq, gk=gk,
            ck=np.ascontiguousarray(ck), sk=np.ascontiguousarray(sk),
            mk=mk, idb=idb))
    return in_maps


def kernel(x, q_w, kv_w, o_w, qnorm_scale, knorm_scale, segment_pos,
           attn_mask, _trace=False):
    import os
    if "nc" not in _module_cache:
        _module_cache["nc"] = _build_module()
    nc = _module_cache["nc"]

    in_maps = _prep_inputs(x, q_w, kv_w, o_w, qnorm_scale, knorm_scale,
                           segment_pos, attn_mask)
    res = run_bass_kernel_spmd(nc, in_maps, core_ids=list(range(NCORES)),
                               trace=_trace,
                               trace_cores=list(range(NCORES)) if _trace
                               else None)
    _module_cache["last_results"] = res

    out = np.zeros((B, T, D), np.float32)
    for c in range(NCORES):
        b, j = divmod(c, NQT)
        out[b, TQ * j:TQ * (j + 1), :] = res.results[c]["out"]
    return out


# revision 23
# speedup vs baseline: 1.0697x; 1.0697x over previous
"""Sliding-window GQA attention (Gemma-style) on 8 Trainium2 NeuronCores.

Sharding: data-parallel over tokens. B=2, T=2048 -> 4096 tokens -> 512
queries per core (core c = 4*b + j handles batch b, queries [512j, 512j+512)).
Each core recomputes k/v for its fixed local window of 1536 tokens
[qs-1024, qs+512) (zero-padded on the left at sequence start), so all 8 cores
run one identical NEFF; per-core differences live entirely in the input data
(sliced x, masks with validity baked in, RoPE tables).

On-chip dataflow (per core):
  phase 1: q/k/v projections with W stationary and x^T moving -> q^T/k^T/v^T
           [H=128 partitions, tokens]; fused RMSNorm (ones-matmul column
           sums + gpsimd partition-broadcast of 1/std) and RoPE (head-dim
           permuted on host so the rotate-half is a quadrant-local
           stream_shuffle); v^T transposed back to [s, h] via PE transposes.
  phase 2: logits^T = k^T.T @ q^T per (head, q-tile) -> tanh softcap + exp on
           ACT -> mask multiply (host-built masks) -> denominators via
           ones-matmul over s-partitions -> P^T @ ... PV accumulation ->
           encT scaled by 1/den on evacuation.
  phase 3: output projection accumulating over heads, DMA straight from PSUM.
"""

import numpy as np
import ml_dtypes

import concourse.bass as bass
import concourse.mybir as mybir
import concourse.tile as tile
from concourse import library_config
from concourse.masks import make_identity
from concourse.bass_utils import run_bass_kernel_spmd

AF = mybir.ActivationFunctionType
ALU = mybir.AluOpType
F32 = mybir.dt.float32
BF16 = mybir.dt.bfloat16
FP8 = mybir.dt.float8e4

B, T, D = 2, 2048, 2048
N, K, H = 16, 8, 128
G = N // K
SOFT_CAP = 50.0
WINDOW = 1024
SCALE = H ** -0.5
ROPE_BASE = 10000.0
EPS = 1e-6

TQ = 512            # queries per core
TKV = 1536          # kv window per core
VST = 129           # per-s-tile width in vsb: 128 v cols + ones column
NQT = TQ // 128     # 4 q-tiles
NST = TKV // 128    # 12 s-tiles
ND = D // 128       # 16 d-tiles
NWIN = 9            # s-tiles in a q-tile's window
NCORES = 8

# quadrant-local half swap for stream_shuffle (32-partition groups)
SWAP16 = list(range(16, 32)) + list(range(16))


def _rope_perm():
    """orig[p] = original head-dim index stored at partition p; freq[p];
    sign[p] for the sin table."""
    orig = np.zeros(128, np.int64)
    freq = np.zeros(128, np.int64)
    sign = np.zeros(128, np.float32)
    for p in range(128):
        qd, o = divmod(p, 32)
        if o < 16:
            orig[p] = 16 * qd + o
            freq[p] = 16 * qd + o
            sign[p] = -1.0
        else:
            orig[p] = 64 + 16 * qd + (o - 16)
            freq[p] = 16 * qd + (o - 16)
            sign[p] = 1.0
    return orig, freq, sign


_ORIG, _FREQ, _SIGN = _rope_perm()

_module_cache = {}

# Instruction types lowered to CTRL encodings: the walrus build in this
# container supports only ONE sync-wait on them ("Too many sync wait
# commands" / "ISA wrong length" in codegen otherwise).
_CTRL_TYPES = ("InstDrain", "InstNoOp", "InstISA", "InstEventSemaphore")


def _split_ctrl_multiwaits(nc, maxw=1):
    """Move excess sem-waits off CTRL-type instructions onto preceding
    same-engine NoOps (same engine queue => identical ordering semantics)."""
    import concourse.mybir as mybir
    for f in nc.m.functions:
        for blk in f.blocks:
            insts = blk.instructions
            out = []
            changed = False
            for inst in insts:
                si = inst.sync_info
                if (si is not None and si.on_wait
                        and len(si.on_wait) > maxw):
                    waits = list(si.on_wait)
                    extra, keep = waits[:-maxw], waits[-maxw:]
                    for k, w in enumerate(extra):
                        nop = mybir.InstNoOp(name=f"{inst.name}-ws{k}",
                                             ins=[], outs=[])
                        nop.engine = inst.engine
                        nop.sync_info = mybir.SyncInfo(on_wait=[w],
                                                       on_update=[])
                        out.append(nop)
                    si.on_wait = keep
                    changed = True
                out.append(inst)
            if changed:
                insts[:] = out


def _build_module(split=True):
    nc = bass.Bass("TRN2", target_bir_lowering=False, debug=False)

    xt_d = nc.dram_tensor("xt", (D, TKV), BF16, kind="ExternalInput").ap()
    wq_d = nc.dram_tensor("wq", (N, D, H), BF16, kind="ExternalInput").ap()
    wk_d = nc.dram_tensor("wk", (K, D, H), BF16, kind="ExternalInput").ap()
    wv_d = nc.dram_tensor("wv", (K, D, H), BF16, kind="ExternalInput").ap()
    wo_d = nc.dram_tensor("wo", (N, H, D), BF16, kind="ExternalInput").ap()
    gq_d = nc.dram_tensor("gq", (H, 1), F32, kind="ExternalInput").ap()
    gk_d = nc.dram_tensor("gk", (H, 1), F32, kind="ExternalInput").ap()
    ck_d = nc.dram_tensor("ck", (H, TKV), BF16, kind="ExternalInput").ap()
    sk_d = nc.dram_tensor("sk", (H, TKV), BF16, kind="ExternalInput").ap()
    mk_d = nc.dram_tensor("mk", (NQT, 128, NWIN * 128), FP8,
                          kind="ExternalInput").ap()
    idb_d = nc.dram_tensor("idb", (128, 128), BF16, kind="ExternalInput").ap()
    out_d = nc.dram_tensor("out", (TQ, D), F32, kind="ExternalOutput").ap()

    with tile.TileContext(nc) as tc:
        with tc.tile_pool(name="const", bufs=1) as cst, \
             tc.tile_pool(name="acc", bufs=1) as acc, \
             tc.tile_pool(name="wst", bufs=2) as wst, \
             tc.tile_pool(name="wost", bufs=3) as wost, \
             tc.tile_pool(name="scr", bufs=2) as scr, \
             tc.tile_pool(name="pp", bufs=3) as pp, \
             tc.tile_pool(name="psA", bufs=4, space="PSUM") as psA, \
             tc.tile_pool(name="psB", bufs=4, space="PSUM") as psB:

            # ---- constants / preloads ----
            xts = cst.tile([128, ND * TKV], BF16, tag="xts")
            xt_r = xt_d.rearrange("(d p) t -> d p t", p=128)
            for d in range(ND):
                nc.sync.dma_start(xts[:, d * TKV:(d + 1) * TKV], xt_r[d])
            gq_t = cst.tile([H, 1], F32, tag="gq")
            nc.sync.dma_start(gq_t[:], gq_d[:])
            gk_t = cst.tile([H, 1], F32, tag="gk")
            nc.sync.dma_start(gk_t[:], gk_d[:])
            ck_t = cst.tile([H, TKV], BF16, tag="ck")
            nc.sync.dma_start(ck_t[:], ck_d[:])
            sk_t = cst.tile([H, TKV], BF16, tag="sk")
            nc.sync.dma_start(sk_t[:], sk_d[:])
            mk_t = cst.tile([128, NQT * NWIN * 128], FP8, tag="mk")
            nc.sync.dma_start(
                mk_t[:].rearrange("p (q m) -> p q m", q=NQT),
                mk_d.rearrange("q p m -> p q m"))
            idb_t = cst.tile([128, 128], BF16, tag="idb")
            nc.sync.dma_start(idb_t[:], idb_d[:])
            ones_bf = cst.tile([128, 1], BF16, tag="ones")
            nc.vector.memset(ones_bf[:], 1.0)
            on1 = cst.tile([1, 128], F32, tag="on1")
            nc.vector.memset(on1[:], 1.0)
            id1 = cst.tile([1, 1], F32, tag="id1")
            nc.vector.memset(id1[:], 1.0)
            eps_t = cst.tile([1, 1], F32, tag="eps")
            nc.vector.memset(eps_t[:], EPS)


            # ---- big accumulators ----
            qTn = acc.tile([128, N * TQ], BF16, tag="qTn")
            kTn = acc.tile([128, K * TKV], BF16, tag="kTn")
            vsb = acc.tile([128, K * NST * VST], BF16, tag="vsb")
            nc.vector.memset(vsb[:], 1.0)
            encT = acc.tile([128, N * NQT * 128], BF16, tag="encT")
            # per-s-tile 1/(50*std_k) columns, [128 s, K*NST]
            rkc = acc.tile([128, K * NST], F32, tag="rkc")

            def rope(src_f32, c0, out_slice):
                rot = scr.tile([128, 512], F32, tag="rot")
                nc.vector.stream_shuffle(rot[:], src_f32[:], SWAP16)
                t1 = scr.tile([128, 512], F32, tag="t1")
                nc.vector.tensor_mul(t1[:], src_f32[:], ck_t[:, c0:c0 + 512])
                t2 = scr.tile([128, 512], F32, tag="t2")
                nc.vector.tensor_mul(t2[:], rot[:], sk_t[:, c0:c0 + 512])
                nc.vector.tensor_add(out_slice, t1[:], t2[:])

            def sumsq_rows(ps):
                """ps [128,512] f32 psum -> std [1,512] f32 = sqrt(ms+eps)."""
                sq = scr.tile([128, 512], BF16, tag="sq")
                nc.scalar.activation(sq[:], ps[:], AF.Square)
                ssp = psA.tile([1, 512], F32, tag="big")
                nc.tensor.matmul(ssp[:], ones_bf[:], sq[:],
                                 start=True, stop=True)
                std = scr.tile([1, 512], F32, tag="row")
                nc.scalar.activation(std[:], ssp[:], AF.Sqrt,
                                     scale=1.0 / H, bias=eps_t[:])
                return std

            # ---- phase 1: q heads (norm fully applied on q) ----
            q_s1 = q_s2 = None
            for n in range(N + 2):
                nstate = None
                if n < N:
                    w_t = wst.tile([128, ND * H], BF16, tag="w")
                    nc.sync.dma_start(
                        w_t[:].rearrange("p (d h) -> p d h", d=ND),
                        wq_d[n].rearrange("(d p) h -> p d h", p=128))
                    ps = psA.tile([128, 512], F32, tag="big")
                    for d in range(ND):
                        nc.tensor.matmul(
                            ps[:], w_t[:, d * H:(d + 1) * H],
                            xts[:, d * TKV + 1024:d * TKV + 1536],
                            start=(d == 0), stop=(d == ND - 1))
                    praw = scr.tile([128, 512], F32, tag="praw")
                    nc.vector.tensor_copy(praw[:], ps[:])
                    sq = scr.tile([128, 512], BF16, tag="sq")
                    nc.scalar.activation(sq[:], ps[:], AF.Square)
                    nstate = (n, praw, sq)
                if q_s1 is not None:
                    n1, praw1, sq1 = q_s1
                    ssp = psA.tile([1, 512], F32, tag="big")
                    nc.tensor.matmul(ssp[:], ones_bf[:], sq1[:],
                                     start=True, stop=True)
                    std = scr.tile([1, 512], F32, tag="row")
                    nc.scalar.activation(std[:], ssp[:], AF.Sqrt,
                                         scale=1.0 / H, bias=eps_t[:])
                    lnt = scr.tile([1, 512], F32, tag="row")
                    nc.scalar.activation(lnt[:], std[:], AF.Ln)
                    rst = scr.tile([1, 512], F32, tag="row")
                    nc.scalar.activation(rst[:], lnt[:], AF.Exp, scale=-1.0)
                    q_s1 = (n1, praw1, rst)
                if q_s2 is not None:
                    n2, praw2, rst2 = q_s2
                    rbp = psA.tile([128, 512], F32, tag="big")
                    nc.tensor.matmul(rbp[:], on1[:], rst2[:],
                                     start=True, stop=True)
                    qn = scr.tile([128, 512], F32, tag="qn")
                    nc.vector.scalar_tensor_tensor(
                        qn[:], praw2[:], gq_t[:], rbp[:],
                        op0=ALU.mult, op1=ALU.mult)
                    rope(qn, 1024, qTn[:, n2 * TQ:(n2 + 1) * TQ])
                q_s2 = q_s1
                q_s1 = nstate
            k_s1 = k_s2 = None
            rkps = {}
            nchunks = K * 3
            for ci in range(nchunks + 2):
                nstate = None
                if ci < nchunks:
                    kh, c = divmod(ci, 3)
                    if c == 0:
                        w_t = wst.tile([128, ND * H], BF16, tag="w")
                        nc.sync.dma_start(
                            w_t[:].rearrange("p (d h) -> p d h", d=ND),
                            wk_d[kh].rearrange("(d p) h -> p d h", p=128))
                        rkps[kh] = psB.tile([128, NST], F32, tag="sm",
                                            name=f"rkp_{kh}")
                    ps = psA.tile([128, 512], F32, tag="big")
                    for d in range(ND):
                        nc.tensor.matmul(
                            ps[:], w_t[:, d * H:(d + 1) * H],
                            xts[:, d * TKV + c * 512:d * TKV + (c + 1) * 512],
                            start=(d == 0), stop=(d == ND - 1))
                    kn = scr.tile([128, 512], F32, tag="kn")
                    nc.vector.tensor_scalar_mul(kn[:], ps[:], gk_t[:])
                    sq = scr.tile([128, 512], BF16, tag="sq")
                    nc.scalar.activation(sq[:], ps[:], AF.Square)
                    nstate = (kh, c, kn, sq)
                if k_s1 is not None:
                    kh1, c1, kn1, sq1 = k_s1
                    ssp = psA.tile([1, 512], F32, tag="big")
                    nc.tensor.matmul(ssp[:], ones_bf[:], sq1[:],
                                     start=True, stop=True)
                    std = scr.tile([1, 512], F32, tag="row")
                    nc.scalar.activation(std[:], ssp[:], AF.Sqrt,
                                         scale=1.0 / H, bias=eps_t[:])
                    k_s1 = (kh1, c1, kn1, std)
                if k_s2 is not None:
                    kh2, c2, kn2, std2 = k_s2
                    rkp2 = rkps[kh2]
                    for t4 in range(4):
                        st = c2 * 4 + t4
                        nc.tensor.matmul(
                            rkp2[:, st:st + 1],
                            std2[:, t4 * 128:(t4 + 1) * 128], id1[:],
                            is_transpose=True, start=True, stop=True)
                    rope(kn2, c2 * 512,
                         kTn[:, kh2 * TKV + c2 * 512:kh2 * TKV + (c2 + 1) * 512])
                    if c2 == 2:
                        rkraw = scr.tile([128, NST], F32, tag="rkraw")
                        nc.scalar.copy(rkraw[:], rkp2[:])
                        nc.vector.reciprocal(
                            rkc[:, kh2 * NST:(kh2 + 1) * NST], rkraw[:])
                        del rkps[kh2]
                k_s2 = k_s1
                k_s1 = nstate
            vstate = None
            for ci in range(nchunks + 1):
                nstate = None
                if ci < nchunks:
                    kh, c = divmod(ci, 3)
                    if c == 0:
                        w_t = wst.tile([128, ND * H], BF16, tag="w")
                        nc.sync.dma_start(
                            w_t[:].rearrange("p (d h) -> p d h", d=ND),
                            wv_d[kh].rearrange("(d p) h -> p d h", p=128))
                    ps = psA.tile([128, 512], F32, tag="big")
                    for d in range(ND):
                        nc.tensor.matmul(
                            ps[:], w_t[:, d * H:(d + 1) * H],
                            xts[:, d * TKV + c * 512:d * TKV + (c + 1) * 512],
                            start=(d == 0), stop=(d == ND - 1))
                    vt_sb = scr.tile([128, 512], BF16, tag="vt")
                    nc.vector.tensor_copy(vt_sb[:], ps[:])
                    nstate = (kh, c, vt_sb)
                if vstate is not None:
                    kh0, c0, vt0 = vstate
                    for t4 in range(4):
                        st = c0 * 4 + t4
                        tps = psB.tile([128, 128], BF16, tag="sm")
                        nc.tensor.matmul(
                            tps[:], vt0[:, t4 * 128:(t4 + 1) * 128],
                            idb_t[:], is_transpose=True,
                            start=True, stop=True)
                        off = (kh0 * NST + st) * VST
                        nc.scalar.copy(vsb[:, off:off + 128], tps[:])
                vstate = nstate

            # ---- phase 2: attention (PV staggered one iteration behind QK) ----
            a_s1 = a_s2 = None
            iters = [(n, qi) for n in range(N) for qi in range(NQT)]
            for it in range(len(iters) + 2):
                nstate = None
                if it < len(iters):
                    n, qi = iters[it]
                    kh = n // G
                    probs = pp.tile([128, NWIN * 128], BF16, tag="probs")
                    for c in range(3):
                        nr = 4 if c < 2 else 1
                        if c < 2:
                            lg = psA.tile([128, 512], F32, tag="big")
                        else:
                            lg = psB.tile([128, 128], F32, tag="sm")
                        for rr in range(nr):
                            r = c * 4 + rr
                            st = qi + r
                            nc.tensor.matmul(
                                lg[:, rr * 128:(rr + 1) * 128],
                                kTn[:, kh * TKV + st * 128:kh * TKV + (st + 1) * 128],
                                qTn[:, n * TQ + qi * 128:n * TQ + (qi + 1) * 128],
                                start=True, stop=True)
                        rk_sl = rkc[:, kh * NST + qi + c * 4:
                                    kh * NST + qi + c * 4 + nr]
                        rk_b = bass.AP(rk_sl.tensor, rk_sl.offset,
                                       list(rk_sl.ap) + [[0, 128]])
                        ttA = scr.tile([128, 512], F32, tag="ttA")
                        nc.vector.tensor_tensor(
                            ttA[:, :nr * 128].rearrange(
                                "p (r t) -> p r t", r=nr),
                            lg[:].rearrange("p (r t) -> p r t", r=nr),
                            rk_b, op=ALU.mult)
                        ttB = scr.tile([128, 512], F32, tag="ttB")
                        nc.scalar.activation(ttB[:, :nr * 128],
                                             ttA[:, :nr * 128],
                                             AF.Tanh, scale=1.0 / SOFT_CAP)
                        ee = scr.tile([128, 512], BF16, tag="ee")
                        nc.scalar.activation(ee[:, :nr * 128],
                                             ttB[:, :nr * 128],
                                             AF.Exp, scale=SOFT_CAP)
                        nc.vector.tensor_mul(
                            probs[:, c * 512:c * 512 + nr * 128],
                            ee[:, :nr * 128],
                            mk_t[:, (qi * NWIN + c * 4) * 128:
                                 (qi * NWIN + c * 4 + nr) * 128])
                    nstate = (n, qi, probs)
                if a_s2 is not None:
                    n0, qi0, probs0 = a_s2
                    kh0 = n0 // G
                    ev = psB.tile([128, VST + 3], F32, tag="sm")
                    for r in range(NWIN):
                        st = qi0 + r
                        off = (kh0 * NST + st) * VST
                        nc.tensor.matmul(
                            ev[:, 0:VST],
                            probs0[:, r * 128:(r + 1) * 128],
                            vsb[:, off:off + VST],
                            start=(r == 0), stop=(r == NWIN - 1))
                    den = scr.tile([128, 1], F32, tag="den")
                    nc.vector.tensor_copy(den[:], ev[:, 128:129])
                    rden = scr.tile([128, 1], F32, tag="rden")
                    nc.vector.reciprocal(rden[:], den[:])
                    enc_sb = scr.tile([128, H], BF16, tag="encsb")
                    nc.vector.tensor_scalar_mul(enc_sb[:], ev[:, 0:H], rden[:])
                    etp = psB.tile([128, 128], BF16, tag="sm")
                    nc.tensor.matmul(etp[:], enc_sb[:], idb_t[:],
                                     is_transpose=True, start=True, stop=True)
                    nc.scalar.copy(
                        encT[:, (n0 * NQT + qi0) * 128:(n0 * NQT + qi0 + 1) * 128],
                        etp[:])
                a_s2 = a_s1
                a_s1 = nstate

            # ---- phase 3: output projection ----
            for dc in range(4):
                ops = [psA.tile([128, 512], F32, tag="big", name=f"op_{dc}_{qi}")
                       for qi in range(NQT)]
                for n in range(N):
                    wo_sl = wost.tile([128, 512], BF16, tag="wo")
                    nc.sync.dma_start(wo_sl[:],
                                      wo_d[n][:, dc * 512:(dc + 1) * 512])
                    for qi in range(NQT):
                        nc.tensor.matmul(
                            ops[qi][:],
                            encT[:, (n * NQT + qi) * 128:(n * NQT + qi + 1) * 128],
                            wo_sl[:], start=(n == 0), stop=(n == N - 1))
                for qi in range(NQT):
                    osb = scr.tile([128, 512], F32, tag="osb")
                    nc.vector.tensor_copy(osb[:], ops[qi][:])
                    nc.sync.dma_start(
                        out_d[qi * 128:(qi + 1) * 128, dc * 512:(dc + 1) * 512],
                        osb[:])

    if split:
        _split_ctrl_multiwaits(nc)
    return nc


def _prep_inputs(x, q_w, kv_w, o_w, qnorm_scale, knorm_scale, segment_pos,
                 attn_mask):
    """Host-side shard + layout prep. Returns list of 8 input dicts."""
    bf = ml_dtypes.bfloat16
    f8 = ml_dtypes.float8_e4m3
    x = np.asarray(x, np.float32)
    q_w = np.asarray(q_w, np.float32)
    kv_w = np.asarray(kv_w, np.float32)
    o_w = np.asarray(o_w, np.float32)
    qnorm_scale = np.asarray(qnorm_scale, np.float32)
    knorm_scale = np.asarray(knorm_scale, np.float32)
    segment_pos = np.asarray(segment_pos, np.int64)
    attn_mask = np.asarray(attn_mask, bool)

    # shared (same array object across cores -> no copy)
    wq = np.ascontiguousarray(q_w[:, :, _ORIG]).astype(bf)
    wk = np.ascontiguousarray(kv_w[0][:, :, _ORIG]).astype(bf)
    wv = kv_w[1].astype(bf)
    wo = o_w.astype(bf)
    gq = ((1.0 + qnorm_scale[_ORIG]) * SCALE).reshape(H, 1).astype(np.float32)
    gk = (1.0 + knorm_scale[_ORIG]).reshape(H, 1).astype(np.float32)
    timescale = ROPE_BASE ** (2.0 * _FREQ.astype(np.float64) / H)  # [128]
    idb = np.eye(128, dtype=bf)

    in_maps = []
    for c in range(NCORES):
        b, j = divmod(c, NQT)
        qs = TQ * j
        kvs = qs - WINDOW

        # x^T for local kv window, zero-padded on the left
        xt = np.zeros((D, TKV), bf)
        lo = max(kvs, 0)
        xt[:, lo - kvs:] = x[b, lo:qs + TQ, :].T.astype(bf)

        # rope tables in permuted row order; positions from segment_pos
        pos = np.zeros(TKV, np.float64)
        pos[lo - kvs:] = segment_pos[b, lo:qs + TQ].astype(np.float64)
        theta = pos[None, :] / timescale[:, None]          # [128, TKV]
        ck = np.cos(theta).astype(bf)
        sk = (np.sin(theta) * _SIGN[:, None]).astype(bf)

        # masks [NQT, 128 s_p, NWIN*128 (r, t)] with validity baked in
        mk = np.zeros((NQT, 128, NWIN * 128), f8)
        seg = segment_pos[b]
        for qi in range(NQT):
            q_glob = qs + qi * 128 + np.arange(128)                  # [t]
            st = qi + np.arange(NWIN)
            k_glob = (kvs + st[:, None] * 128 + np.arange(128)[None, :])
            valid = k_glob >= 0                                       # [r, sp]
            k_safe = np.clip(k_glob, 0, T - 1)
            am = attn_mask[b][np.ix_(q_glob, k_safe.reshape(-1))]     # [t, r*sp]
            pk = seg[k_safe.reshape(-1)]                              # [r*sp]
            pq = seg[q_glob]                                          # [t]
            win = ((pk[None, :] > pq[:, None] - WINDOW)
                   & (pk[None, :] < pq[:, None] + WINDOW))
            m = (am & win & valid.reshape(1, -1)).astype(np.float32)  # [t, r*sp]
            m = m.reshape(128, NWIN, 128).transpose(2, 1, 0)          # [sp, r, t]
            mk[qi] = m.reshape(128, NWIN * 128).astype(f8)

        in_maps.append(dict(
            xt=xt, wq=wq, wk=wk, wv=wv, wo=wo, gq=gq, gk=gk,
            ck=np.ascontiguousarray(ck), sk=np.ascontiguousarray(sk),
            mk=mk, idb=idb))
    return in_maps


def kernel(x, q_w, kv_w, o_w, qnorm_scale, knorm_scale, segment_pos,
           attn_mask, _trace=False):
    import os
    if "nc" not in _module_cache:
        _module_cache["nc"] = _build_module()
    nc = _module_cache["nc"]

    in_maps = _prep_inputs(x, q_w, kv_w, o_w, qnorm_scale, knorm_scale,
                           segment_pos, attn_mask)
    res = run_bass_kernel_spmd(nc, in_maps, core_ids=list(range(NCORES)),
                               trace=_trace,
                               trace_cores=list(range(NCORES)) if _trace
                               else None)
    _module_cache["last_results"] = res

    out = np.zeros((B, T, D), np.float32)
    for c in range(NCORES):
        b, j = divmod(c, NQT)
        out[b, TQ * j:TQ * (j + 1), :] = res.results[c]["out"]
    return out


# revision 24
# speedup vs baseline: 1.0990x; 1.0273x over previous
"""Sliding-window GQA attention (Gemma-style) on 8 Trainium2 NeuronCores.

Sharding: data-parallel over tokens. B=2, T=2048 -> 4096 tokens -> 512
queries per core (core c = 4*b + j handles batch b, queries [512j, 512j+512)).
Each core recomputes k/v for its fixed local window of 1536 tokens
[qs-1024, qs+512) (zero-padded on the left at sequence start), so all 8 cores
run one identical NEFF; per-core differences live entirely in the input data
(sliced x, masks with validity baked in, RoPE tables).

On-chip dataflow (per core):
  phase 1: q/k/v projections with W stationary and x^T moving -> q^T/k^T/v^T
           [H=128 partitions, tokens]; fused RMSNorm (ones-matmul column
           sums + gpsimd partition-broadcast of 1/std) and RoPE (head-dim
           permuted on host so the rotate-half is a quadrant-local
           stream_shuffle); v^T transposed back to [s, h] via PE transposes.
  phase 2: logits^T = k^T.T @ q^T per (head, q-tile) -> tanh softcap + exp on
           ACT -> mask multiply (host-built masks) -> denominators via
           ones-matmul over s-partitions -> P^T @ ... PV accumulation ->
           encT scaled by 1/den on evacuation.
  phase 3: output projection accumulating over heads, DMA straight from PSUM.
"""

import numpy as np
import ml_dtypes

import concourse.bass as bass
import concourse.mybir as mybir
import concourse.tile as tile
from concourse import library_config
from concourse.masks import make_identity
from concourse.bass_utils import run_bass_kernel_spmd

AF = mybir.ActivationFunctionType
ALU = mybir.AluOpType
F32 = mybir.dt.float32
BF16 = mybir.dt.bfloat16
FP8 = mybir.dt.float8e4

B, T, D = 2, 2048, 2048
N, K, H = 16, 8, 128
G = N // K
SOFT_CAP = 50.0
WINDOW = 1024
SCALE = H ** -0.5
ROPE_BASE = 10000.0
EPS = 1e-6

TQ = 512            # queries per core
TKV = 1536          # kv window per core
VST = 129           # per-s-tile width in vsb: 128 v cols + ones column
NQT = TQ // 128     # 4 q-tiles
NST = TKV // 128    # 12 s-tiles
ND = D // 128       # 16 d-tiles
NWIN = 9            # s-tiles in a q-tile's window
NCORES = 8

# quadrant-local half swap for stream_shuffle (32-partition groups)
SWAP16 = list(range(16, 32)) + list(range(16))


def _rope_perm():
    """orig[p] = original head-dim index stored at partition p; freq[p];
    sign[p] for the sin table."""
    orig = np.zeros(128, np.int64)
    freq = np.zeros(128, np.int64)
    sign = np.zeros(128, np.float32)
    for p in range(128):
        qd, o = divmod(p, 32)
        if o < 16:
            orig[p] = 16 * qd + o
            freq[p] = 16 * qd + o
            sign[p] = -1.0
        else:
            orig[p] = 64 + 16 * qd + (o - 16)
            freq[p] = 16 * qd + (o - 16)
            sign[p] = 1.0
    return orig, freq, sign


_ORIG, _FREQ, _SIGN = _rope_perm()

_module_cache = {}

# Instruction types lowered to CTRL encodings: the walrus build in this
# container supports only ONE sync-wait on them ("Too many sync wait
# commands" / "ISA wrong length" in codegen otherwise).
_CTRL_TYPES = ("InstDrain", "InstNoOp", "InstISA", "InstEventSemaphore")


def _split_ctrl_multiwaits(nc, maxw=1):
    """Move excess sem-waits off CTRL-type instructions onto preceding
    same-engine NoOps (same engine queue => identical ordering semantics)."""
    import concourse.mybir as mybir
    for f in nc.m.functions:
        for blk in f.blocks:
            insts = blk.instructions
            out = []
            changed = False
            for inst in insts:
                si = inst.sync_info
                if (si is not None and si.on_wait
                        and len(si.on_wait) > maxw):
                    waits = list(si.on_wait)
                    extra, keep = waits[:-maxw], waits[-maxw:]
                    for k, w in enumerate(extra):
                        nop = mybir.InstNoOp(name=f"{inst.name}-ws{k}",
                                             ins=[], outs=[])
                        nop.engine = inst.engine
                        nop.sync_info = mybir.SyncInfo(on_wait=[w],
                                                       on_update=[])
                        out.append(nop)
                    si.on_wait = keep
                    changed = True
                out.append(inst)
            if changed:
                insts[:] = out


def _build_module(split=True):
    nc = bass.Bass("TRN2", target_bir_lowering=False, debug=False)

    xt_d = nc.dram_tensor("xt", (D, TKV), BF16, kind="ExternalInput").ap()
    wq_d = nc.dram_tensor("wq", (N, D, H), BF16, kind="ExternalInput").ap()
    wk_d = nc.dram_tensor("wk", (K, D, H), BF16, kind="ExternalInput").ap()
    wv_d = nc.dram_tensor("wv", (K, D, H), BF16, kind="ExternalInput").ap()
    wo_d = nc.dram_tensor("wo", (N, H, D), BF16, kind="ExternalInput").ap()
    gq_d = nc.dram_tensor("gq", (H, 1), F32, kind="ExternalInput").ap()
    gk_d = nc.dram_tensor("gk", (H, 1), F32, kind="ExternalInput").ap()
    ck_d = nc.dram_tensor("ck", (H, TKV), BF16, kind="ExternalInput").ap()
    sk_d = nc.dram_tensor("sk", (H, TKV), BF16, kind="ExternalInput").ap()
    mk_d = nc.dram_tensor("mk", (NQT, 128, NWIN * 128), FP8,
                          kind="ExternalInput").ap()
    idb_d = nc.dram_tensor("idb", (128, 128), BF16, kind="ExternalInput").ap()
    out_d = nc.dram_tensor("out", (TQ, D), F32, kind="ExternalOutput").ap()

    with tile.TileContext(nc) as tc:
        with tc.tile_pool(name="const", bufs=1) as cst, \
             tc.tile_pool(name="acc", bufs=1) as acc, \
             tc.tile_pool(name="wst", bufs=2) as wst, \
             tc.tile_pool(name="wost", bufs=3) as wost, \
             tc.tile_pool(name="scr", bufs=2) as scr, \
             tc.tile_pool(name="pp", bufs=3) as pp, \
             tc.tile_pool(name="psA", bufs=4, space="PSUM") as psA, \
             tc.tile_pool(name="psB", bufs=4, space="PSUM") as psB:

            # ---- constants / preloads ----
            xts = cst.tile([128, ND * TKV], BF16, tag="xts")
            xt_r = xt_d.rearrange("(d p) t -> d p t", p=128)
            for d in range(ND):
                nc.sync.dma_start(xts[:, d * TKV:(d + 1) * TKV], xt_r[d])
            gq_t = cst.tile([H, 1], F32, tag="gq")
            nc.sync.dma_start(gq_t[:], gq_d[:])
            gk_t = cst.tile([H, 1], F32, tag="gk")
            nc.sync.dma_start(gk_t[:], gk_d[:])
            ck_t = cst.tile([H, TKV], BF16, tag="ck")
            nc.sync.dma_start(ck_t[:], ck_d[:])
            sk_t = cst.tile([H, TKV], BF16, tag="sk")
            nc.sync.dma_start(sk_t[:], sk_d[:])
            mk_t = cst.tile([128, NQT * NWIN * 128], FP8, tag="mk")
            nc.sync.dma_start(
                mk_t[:].rearrange("p (q m) -> p q m", q=NQT),
                mk_d.rearrange("q p m -> p q m"))
            idb_t = cst.tile([128, 128], BF16, tag="idb")
            nc.sync.dma_start(idb_t[:], idb_d[:])
            ones_bf = cst.tile([128, 1], BF16, tag="ones")
            nc.vector.memset(ones_bf[:], 1.0)
            on1 = cst.tile([1, 128], F32, tag="on1")
            nc.vector.memset(on1[:], 1.0)
            id1 = cst.tile([1, 1], F32, tag="id1")
            nc.vector.memset(id1[:], 1.0)
            eps_t = cst.tile([1, 1], F32, tag="eps")
            nc.vector.memset(eps_t[:], EPS)


            # ---- big accumulators ----
            qTn = acc.tile([128, N * TQ], BF16, tag="qTn")
            kTn = acc.tile([128, K * TKV], BF16, tag="kTn")
            vsb = acc.tile([128, K * NST * VST], BF16, tag="vsb")
            nc.vector.memset(vsb[:], 1.0)
            encT = acc.tile([128, N * NQT * 128], BF16, tag="encT")
            # per-s-tile 1/(50*std_k) columns, [128 s, K*NST]
            rkc = acc.tile([128, K * NST], F32, tag="rkc")

            def rope(src_f32, c0, out_slice):
                rot = scr.tile([128, 512], F32, tag="rot")
                nc.vector.stream_shuffle(rot[:], src_f32[:], SWAP16)
                t1 = scr.tile([128, 512], F32, tag="t1")
                nc.vector.tensor_mul(t1[:], src_f32[:], ck_t[:, c0:c0 + 512])
                t2 = scr.tile([128, 512], F32, tag="t2")
                nc.vector.tensor_mul(t2[:], rot[:], sk_t[:, c0:c0 + 512])
                nc.vector.tensor_add(out_slice, t1[:], t2[:])

            def sumsq_rows(ps):
                """ps [128,512] f32 psum -> std [1,512] f32 = sqrt(ms+eps)."""
                sq = scr.tile([128, 512], BF16, tag="sq")
                nc.scalar.activation(sq[:], ps[:], AF.Square)
                ssp = psA.tile([1, 512], F32, tag="big")
                nc.tensor.matmul(ssp[:], ones_bf[:], sq[:],
                                 start=True, stop=True)
                std = scr.tile([1, 512], F32, tag="row")
                nc.scalar.activation(std[:], ssp[:], AF.Sqrt,
                                     scale=1.0 / H, bias=eps_t[:])
                return std

            # ---- phase 1: q heads (norm fully applied on q) ----
            q_s1 = q_s2 = None
            for n in range(N + 2):
                nstate = None
                if n < N:
                    w_t = wst.tile([128, ND * H], BF16, tag="w")
                    nc.sync.dma_start(
                        w_t[:].rearrange("p (d h) -> p d h", d=ND),
                        wq_d[n].rearrange("(d p) h -> p d h", p=128))
                    ps = psA.tile([128, 512], F32, tag="big")
                    for d in range(ND):
                        nc.tensor.matmul(
                            ps[:], w_t[:, d * H:(d + 1) * H],
                            xts[:, d * TKV + 1024:d * TKV + 1536],
                            start=(d == 0), stop=(d == ND - 1))
                    praw = scr.tile([128, 512], F32, tag="praw")
                    nc.vector.tensor_copy(praw[:], ps[:])
                    sq = scr.tile([128, 512], BF16, tag="sq")
                    nc.scalar.activation(sq[:], ps[:], AF.Square)
                    nstate = (n, praw, sq)
                if q_s1 is not None:
                    n1, praw1, sq1 = q_s1
                    ssp = psA.tile([1, 512], F32, tag="big")
                    nc.tensor.matmul(ssp[:], ones_bf[:], sq1[:],
                                     start=True, stop=True)
                    std = scr.tile([1, 512], F32, tag="row")
                    nc.scalar.activation(std[:], ssp[:], AF.Sqrt,
                                         scale=1.0 / H, bias=eps_t[:])
                    lnt = scr.tile([1, 512], F32, tag="row")
                    nc.scalar.activation(lnt[:], std[:], AF.Ln)
                    rst = scr.tile([1, 512], F32, tag="row")
                    nc.scalar.activation(rst[:], lnt[:], AF.Exp, scale=-1.0)
                    q_s1 = (n1, praw1, rst)
                if q_s2 is not None:
                    n2, praw2, rst2 = q_s2
                    rbp = psA.tile([128, 512], F32, tag="big")
                    nc.tensor.matmul(rbp[:], on1[:], rst2[:],
                                     start=True, stop=True)
                    qn = scr.tile([128, 512], F32, tag="qn")
                    nc.vector.scalar_tensor_tensor(
                        qn[:], praw2[:], gq_t[:], rbp[:],
                        op0=ALU.mult, op1=ALU.mult)
                    rope(qn, 1024, qTn[:, n2 * TQ:(n2 + 1) * TQ])
                q_s2 = q_s1
                q_s1 = nstate
            k_s1 = k_s2 = None
            rkps = {}
            nchunks = K * 3
            for ci in range(nchunks + 2):
                nstate = None
                if ci < nchunks:
                    kh, c = divmod(ci, 3)
                    if c == 0:
                        w_t = wst.tile([128, ND * H], BF16, tag="w")
                        nc.sync.dma_start(
                            w_t[:].rearrange("p (d h) -> p d h", d=ND),
                            wk_d[kh].rearrange("(d p) h -> p d h", p=128))
                        rkps[kh] = psB.tile([128, NST], F32, tag="sm",
                                            name=f"rkp_{kh}")
                    ps = psA.tile([128, 512], F32, tag="big")
                    for d in range(ND):
                        nc.tensor.matmul(
                            ps[:], w_t[:, d * H:(d + 1) * H],
                            xts[:, d * TKV + c * 512:d * TKV + (c + 1) * 512],
                            start=(d == 0), stop=(d == ND - 1))
                    kn = scr.tile([128, 512], F32, tag="kn")
                    nc.vector.tensor_scalar_mul(kn[:], ps[:], gk_t[:])
                    sq = scr.tile([128, 512], BF16, tag="sq")
                    nc.scalar.activation(sq[:], ps[:], AF.Square)
                    nstate = (kh, c, kn, sq)
                if k_s1 is not None:
                    kh1, c1, kn1, sq1 = k_s1
                    ssp = psA.tile([1, 512], F32, tag="big")
                    nc.tensor.matmul(ssp[:], ones_bf[:], sq1[:],
                                     start=True, stop=True)
                    std = scr.tile([1, 512], F32, tag="row")
                    nc.scalar.activation(std[:], ssp[:], AF.Sqrt,
                                         scale=1.0 / H, bias=eps_t[:])
                    k_s1 = (kh1, c1, kn1, std)
                if k_s2 is not None:
                    kh2, c2, kn2, std2 = k_s2
                    rkp2 = rkps[kh2]
                    for t4 in range(4):
                        st = c2 * 4 + t4
                        nc.tensor.matmul(
                            rkp2[:, st:st + 1],
                            std2[:, t4 * 128:(t4 + 1) * 128], id1[:],
                            is_transpose=True, start=True, stop=True)
                    rope(kn2, c2 * 512,
                         kTn[:, kh2 * TKV + c2 * 512:kh2 * TKV + (c2 + 1) * 512])
                    if c2 == 2:
                        rkraw = scr.tile([128, NST], F32, tag="rkraw")
                        nc.scalar.copy(rkraw[:], rkp2[:])
                        nc.vector.reciprocal(
                            rkc[:, kh2 * NST:(kh2 + 1) * NST], rkraw[:])
                        del rkps[kh2]
                k_s2 = k_s1
                k_s1 = nstate
            a_s1 = a_s2 = None
            iters = [(n, qi) for n in range(N) for qi in range(NQT)]
            a_it = [0]

            def attn_step(it):
                nstate = None
                if it < len(iters):
                        n, qi = iters[it]
                        kh = n // G
                        probs = pp.tile([128, NWIN * 128], BF16, tag="probs")
                        for c in range(3):
                            nr = 4 if c < 2 else 1
                            if c < 2:
                                lg = psA.tile([128, 512], F32, tag="big")
                            else:
                                lg = psB.tile([128, 128], F32, tag="sm")
                            for rr in range(nr):
                                r = c * 4 + rr
                                st = qi + r
                                nc.tensor.matmul(
                                    lg[:, rr * 128:(rr + 1) * 128],
                                    kTn[:, kh * TKV + st * 128:kh * TKV + (st + 1) * 128],
                                    qTn[:, n * TQ + qi * 128:n * TQ + (qi + 1) * 128],
                                    start=True, stop=True)
                            rk_sl = rkc[:, kh * NST + qi + c * 4:
                                        kh * NST + qi + c * 4 + nr]
                            rk_b = bass.AP(rk_sl.tensor, rk_sl.offset,
                                           list(rk_sl.ap) + [[0, 128]])
                            ttA = scr.tile([128, 512], F32, tag="ttA")
                            nc.vector.tensor_tensor(
                                ttA[:, :nr * 128].rearrange(
                                    "p (r t) -> p r t", r=nr),
                                lg[:].rearrange("p (r t) -> p r t", r=nr),
                                rk_b, op=ALU.mult)
                            ttB = scr.tile([128, 512], F32, tag="ttB")
                            nc.scalar.activation(ttB[:, :nr * 128],
                                                 ttA[:, :nr * 128],
                                                 AF.Tanh, scale=1.0 / SOFT_CAP)
                            ee = scr.tile([128, 512], BF16, tag="ee")
                            nc.scalar.activation(ee[:, :nr * 128],
                                                 ttB[:, :nr * 128],
                                                 AF.Exp, scale=SOFT_CAP)
                            nc.vector.tensor_mul(
                                probs[:, c * 512:c * 512 + nr * 128],
                                ee[:, :nr * 128],
                                mk_t[:, (qi * NWIN + c * 4) * 128:
                                     (qi * NWIN + c * 4 + nr) * 128])
                        nstate = (n, qi, probs)
                if a_s2 is not None:
                        n0, qi0, probs0 = a_s2
                        kh0 = n0 // G
                        ev = psB.tile([128, VST + 3], F32, tag="sm")
                        for r in range(NWIN):
                            st = qi0 + r
                            off = (kh0 * NST + st) * VST
                            nc.tensor.matmul(
                                ev[:, 0:VST],
                                probs0[:, r * 128:(r + 1) * 128],
                                vsb[:, off:off + VST],
                                start=(r == 0), stop=(r == NWIN - 1))
                        den = scr.tile([128, 1], F32, tag="den")
                        nc.vector.tensor_copy(den[:], ev[:, 128:129])
                        rden = scr.tile([128, 1], F32, tag="rden")
                        nc.vector.reciprocal(rden[:], den[:])
                        enc_sb = scr.tile([128, H], BF16, tag="encsb")
                        nc.vector.tensor_scalar_mul(enc_sb[:], ev[:, 0:H], rden[:])
                        etp = psB.tile([128, 128], BF16, tag="sm")
                        nc.tensor.matmul(etp[:], enc_sb[:], idb_t[:],
                                         is_transpose=True, start=True, stop=True)
                        nc.scalar.copy(
                            encT[:, (n0 * NQT + qi0) * 128:(n0 * NQT + qi0 + 1) * 128],
                            etp[:])
                return nstate

            def attn_advance(k_steps):
                nonlocal a_s1, a_s2
                for _ in range(k_steps):
                    if a_it[0] >= len(iters) + 2:
                        return
                    ns = attn_step(a_it[0])
                    a_it[0] += 1
                    a_s2 = a_s1
                    a_s1 = ns

            vstate = None
            for ci in range(nchunks + 1):
                nstate = None
                if ci < nchunks:
                    kh, c = divmod(ci, 3)
                    if c == 0:
                        w_t = wst.tile([128, ND * H], BF16, tag="w")
                        nc.sync.dma_start(
                            w_t[:].rearrange("p (d h) -> p d h", d=ND),
                            wv_d[kh].rearrange("(d p) h -> p d h", p=128))
                    ps = psA.tile([128, 512], F32, tag="big")
                    for d in range(ND):
                        nc.tensor.matmul(
                            ps[:], w_t[:, d * H:(d + 1) * H],
                            xts[:, d * TKV + c * 512:d * TKV + (c + 1) * 512],
                            start=(d == 0), stop=(d == ND - 1))
                    vt_sb = scr.tile([128, 512], BF16, tag="vt")
                    nc.vector.tensor_copy(vt_sb[:], ps[:])
                    nstate = (kh, c, vt_sb)
                if vstate is not None:
                    kh0, c0, vt0 = vstate
                    for t4 in range(4):
                        st = c0 * 4 + t4
                        tps = psB.tile([128, 128], BF16, tag="sm")
                        nc.tensor.matmul(
                            tps[:], vt0[:, t4 * 128:(t4 + 1) * 128],
                            idb_t[:], is_transpose=True,
                            start=True, stop=True)
                        off = (kh0 * NST + st) * VST
                        nc.scalar.copy(vsb[:, off:off + 128], tps[:])
                    if c0 == 2:
                        # v head kh0 complete: release its attention iters
                        attn_advance(2 * NQT)
                vstate = nstate
            attn_advance(len(iters) + 2 - a_it[0])

            # ---- phase 3: output projection ----
            for dc in range(4):
                ops = [psA.tile([128, 512], F32, tag="big", name=f"op_{dc}_{qi}")
                       for qi in range(NQT)]
                for n in range(N):
                    wo_sl = wost.tile([128, 512], BF16, tag="wo")
                    nc.sync.dma_start(wo_sl[:],
                                      wo_d[n][:, dc * 512:(dc + 1) * 512])
                    for qi in range(NQT):
                        nc.tensor.matmul(
                            ops[qi][:],
                            encT[:, (n * NQT + qi) * 128:(n * NQT + qi + 1) * 128],
                            wo_sl[:], start=(n == 0), stop=(n == N - 1))
                for qi in range(NQT):
                    osb = scr.tile([128, 512], F32, tag="osb")
                    nc.vector.tensor_copy(osb[:], ops[qi][:])
                    nc.sync.dma_start(
                        out_d[qi * 128:(qi + 1) * 128, dc * 512:(dc + 1) * 512],
                        osb[:])

    if split:
        _split_ctrl_multiwaits(nc)
    return nc


def _prep_inputs(x, q_w, kv_w, o_w, qnorm_scale, knorm_scale, segment_pos,
                 attn_mask):
    """Host-side shard + layout prep. Returns list of 8 input dicts."""
    bf = ml_dtypes.bfloat16
    f8 = ml_dtypes.float8_e4m3
    x = np.asarray(x, np.float32)
    q_w = np.asarray(q_w, np.float32)
    kv_w = np.asarray(kv_w, np.float32)
    o_w = np.asarray(o_w, np.float32)
    qnorm_scale = np.asarray(qnorm_scale, np.float32)
    knorm_scale = np.asarray(knorm_scale, np.float32)
    segment_pos = np.asarray(segment_pos, np.int64)
    attn_mask = np.asarray(attn_mask, bool)

    # shared (same array object across cores -> no copy)
    wq = np.ascontiguousarray(q_w[:, :, _ORIG]).astype(bf)
    wk = np.ascontiguousarray(kv_w[0][:, :, _ORIG]).astype(bf)
    wv = kv_w[1].astype(bf)
    wo = o_w.astype(bf)
    gq = ((1.0 + qnorm_scale[_ORIG]) * SCALE).reshape(H, 1).astype(np.float32)
    gk = (1.0 + knorm_scale[_ORIG]).reshape(H, 1).astype(np.float32)
    timescale = ROPE_BASE ** (2.0 * _FREQ.astype(np.float64) / H)  # [128]
    idb = np.eye(128, dtype=bf)

    in_maps = []
    for c in range(NCORES):
        b, j = divmod(c, NQT)
        qs = TQ * j
        kvs = qs - WINDOW

        # x^T for local kv window, zero-padded on the left
        xt = np.zeros((D, TKV), bf)
        lo = max(kvs, 0)
        xt[:, lo - kvs:] = x[b, lo:qs + TQ, :].T.astype(bf)

        # rope tables in permuted row order; positions from segment_pos
        pos = np.zeros(TKV, np.float64)
        pos[lo - kvs:] = segment_pos[b, lo:qs + TQ].astype(np.float64)
        theta = pos[None, :] / timescale[:, None]          # [128, TKV]
        ck = np.cos(theta).astype(bf)
        sk = (np.sin(theta) * _SIGN[:, None]).astype(bf)

        # masks [NQT, 128 s_p, NWIN*128 (r, t)] with validity baked in
        mk = np.zeros((NQT, 128, NWIN * 128), f8)
        seg = segment_pos[b]
        for qi in range(NQT):
            q_glob = qs + qi * 128 + np.arange(128)                  # [t]
            st = qi + np.arange(NWIN)
            k_glob = (kvs + st[:, None] * 128 + np.arange(128)[None, :])
            valid = k_glob >= 0                                       # [r, sp]
            k_safe = np.clip(k_glob, 0, T - 1)
            am = attn_mask[b][np.ix_(q_glob, k_safe.reshape(-1))]     # [t, r*sp]
            pk = seg[k_safe.reshape(-1)]                              # [r*sp]
            pq = seg[q_glob]                                          # [t]
            win = ((pk[None, :] > pq[:, None] - WINDOW)
                   & (pk[None, :] < pq[:, None] + WINDOW))
            m = (am & win & valid.reshape(1, -1)).astype(np.float32)  # [t, r*sp]
            m = m.reshape(128, NWIN, 128).transpose(2, 1, 0)          # [sp, r, t]
            mk[qi] = m.reshape(128, NWIN * 128).astype(f8)

        in_maps.append(dict(
            xt=xt, wq=wq, wk=wk, wv=wv, wo=wo, gq=gq, gk=gk,
            ck=np.ascontiguousarray(ck), sk=np.ascontiguousarray(sk),
            mk=mk, idb=idb))
    return in_maps


def kernel(x, q_w, kv_w, o_w, qnorm_scale, knorm_scale, segment_pos,
           attn_mask, _trace=False):
    import os
    if "nc" not in _module_cache:
        _module_cache["nc"] = _build_module()
    nc = _module_cache["nc"]

    in_maps = _prep_inputs(x, q_w, kv_w, o_w, qnorm_scale, knorm_scale,
                           segment_pos, attn_mask)
    res = run_bass_kernel_spmd(nc, in_maps, core_ids=list(range(NCORES)),
                               trace=_trace,
                               trace_cores=list(range(NCORES)) if _trace
                               else None)
    _module_cache["last_results"] = res

    out = np.zeros((B, T, D), np.float32)
    for c in range(NCORES):
        b, j = divmod(c, NQT)
        out[b, TQ * j:TQ * (j + 1), :] = res.results[c]["out"]
    return out


# revision 27
# speedup vs baseline: 1.1336x; 1.0315x over previous
"""Sliding-window GQA attention (Gemma-style) on 8 Trainium2 NeuronCores.

Sharding: data-parallel over tokens. B=2, T=2048 -> 4096 tokens -> 512
queries per core (core c = 4*b + j handles batch b, queries [512j, 512j+512)).
Each core recomputes k/v for its fixed local window of 1536 tokens
[qs-1024, qs+512) (zero-padded on the left at sequence start), so all 8 cores
run one identical NEFF; per-core differences live entirely in the input data
(sliced x, masks with validity baked in, RoPE tables).

On-chip dataflow (per core):
  phase 1: q/k/v projections with W stationary and x^T moving -> q^T/k^T/v^T
           [H=128 partitions, tokens]; fused RMSNorm (ones-matmul column
           sums + gpsimd partition-broadcast of 1/std) and RoPE (head-dim
           permuted on host so the rotate-half is a quadrant-local
           stream_shuffle); v^T transposed back to [s, h] via PE transposes.
  phase 2: logits^T = k^T.T @ q^T per (head, q-tile) -> tanh softcap + exp on
           ACT -> mask multiply (host-built masks) -> denominators via
           ones-matmul over s-partitions -> P^T @ ... PV accumulation ->
           encT scaled by 1/den on evacuation.
  phase 3: output projection accumulating over heads, DMA straight from PSUM.
"""

import numpy as np
import ml_dtypes

import concourse.bass as bass
import concourse.mybir as mybir
import concourse.tile as tile
from concourse import library_config
from concourse.masks import make_identity
from concourse.bass_utils import run_bass_kernel_spmd

AF = mybir.ActivationFunctionType
ALU = mybir.AluOpType
F32 = mybir.dt.float32
BF16 = mybir.dt.bfloat16
FP8 = mybir.dt.float8e4

B, T, D = 2, 2048, 2048
N, K, H = 16, 8, 128
G = N // K
SOFT_CAP = 50.0
WINDOW = 1024
SCALE = H ** -0.5
ROPE_BASE = 10000.0
EPS = 1e-6

TQ = 512            # queries per core
TKV = 1536          # kv window per core
VST = 129           # per-s-tile width in vsb: 128 v cols + ones column
NQT = TQ // 128     # 4 q-tiles
NST = TKV // 128    # 12 s-tiles
ND = D // 128       # 16 d-tiles
NWIN = 9            # s-tiles in a q-tile's window
NCORES = 8

# quadrant-local half swap for stream_shuffle (32-partition groups)
SWAP16 = list(range(16, 32)) + list(range(16))


def _rope_perm():
    """orig[p] = original head-dim index stored at partition p; freq[p];
    sign[p] for the sin table."""
    orig = np.zeros(128, np.int64)
    freq = np.zeros(128, np.int64)
    sign = np.zeros(128, np.float32)
    for p in range(128):
        qd, o = divmod(p, 32)
        if o < 16:
            orig[p] = 16 * qd + o
            freq[p] = 16 * qd + o
            sign[p] = -1.0
        else:
            orig[p] = 64 + 16 * qd + (o - 16)
            freq[p] = 16 * qd + (o - 16)
            sign[p] = 1.0
    return orig, freq, sign


_ORIG, _FREQ, _SIGN = _rope_perm()

_module_cache = {}

# Instruction types lowered to CTRL encodings: the walrus build in this
# container supports only ONE sync-wait on them ("Too many sync wait
# commands" / "ISA wrong length" in codegen otherwise).
_CTRL_TYPES = ("InstDrain", "InstNoOp", "InstISA", "InstEventSemaphore")


def _split_ctrl_multiwaits(nc, maxw=1):
    """Move excess sem-waits off CTRL-type instructions onto preceding
    same-engine NoOps (same engine queue => identical ordering semantics)."""
    import concourse.mybir as mybir
    for f in nc.m.functions:
        for blk in f.blocks:
            insts = blk.instructions
            out = []
            changed = False
            for inst in insts:
                si = inst.sync_info
                if (si is not None and si.on_wait
                        and len(si.on_wait) > maxw):
                    waits = list(si.on_wait)
                    extra, keep = waits[:-maxw], waits[-maxw:]
                    for k, w in enumerate(extra):
                        nop = mybir.InstNoOp(name=f"{inst.name}-ws{k}",
                                             ins=[], outs=[])
                        nop.engine = inst.engine
                        nop.sync_info = mybir.SyncInfo(on_wait=[w],
                                                       on_update=[])
                        out.append(nop)
                    si.on_wait = keep
                    changed = True
                out.append(inst)
            if changed:
                insts[:] = out


def _build_module(split=True):
    nc = bass.Bass("TRN2", target_bir_lowering=False, debug=False)

    xt_d = nc.dram_tensor("xt", (D, TKV), BF16, kind="ExternalInput").ap()
    wq_d = nc.dram_tensor("wq", (N, D, H), BF16, kind="ExternalInput").ap()
    wk_d = nc.dram_tensor("wk", (K, D, H), BF16, kind="ExternalInput").ap()
    wv_d = nc.dram_tensor("wv", (K, D, H), BF16, kind="ExternalInput").ap()
    wo_d = nc.dram_tensor("wo", (N, H, D), BF16, kind="ExternalInput").ap()
    gq_d = nc.dram_tensor("gq", (H, 1), F32, kind="ExternalInput").ap()
    gk_d = nc.dram_tensor("gk", (H, 1), F32, kind="ExternalInput").ap()
    ck_d = nc.dram_tensor("ck", (H, TKV), BF16, kind="ExternalInput").ap()
    sk_d = nc.dram_tensor("sk", (H, TKV), BF16, kind="ExternalInput").ap()
    mk_d = nc.dram_tensor("mk", (NQT, 128, NWIN * 128), FP8,
                          kind="ExternalInput").ap()
    idb_d = nc.dram_tensor("idb", (128, 128), BF16, kind="ExternalInput").ap()
    out_d = nc.dram_tensor("out", (TQ, D), F32, kind="ExternalOutput").ap()

    with tile.TileContext(nc) as tc:
        with tc.tile_pool(name="const", bufs=1) as cst, \
             tc.tile_pool(name="acc", bufs=1) as acc, \
             tc.tile_pool(name="wst", bufs=2) as wst, \
             tc.tile_pool(name="wost", bufs=3) as wost, \
             tc.tile_pool(name="scr", bufs=2) as scr, \
             tc.tile_pool(name="pp", bufs=3) as pp, \
             tc.tile_pool(name="psA", bufs=4, space="PSUM") as psA, \
             tc.tile_pool(name="psB", bufs=4, space="PSUM") as psB:

            # ---- constants / preloads ----
            # first two q-heads' weights load before the big xts transfer so
            # the first projection matmuls start as soon as x^T tiles land
            w_pre = {}
            for n0 in range(2):
                wp = wst.tile([128, ND * H], BF16, tag="w", name=f"w_pre{n0}")
                nc.sync.dma_start(
                    wp[:].rearrange("p (d h) -> p d h", d=ND),
                    wq_d[n0].rearrange("(d p) h -> p d h", p=128))
                w_pre[n0] = wp
            xts = cst.tile([128, ND * TKV], BF16, tag="xts")
            xt_r = xt_d.rearrange("(d p) t -> d p t", p=128)
            for d in range(ND):
                nc.sync.dma_start(xts[:, d * TKV:(d + 1) * TKV], xt_r[d])
            gq_t = cst.tile([H, 1], F32, tag="gq")
            nc.sync.dma_start(gq_t[:], gq_d[:])
            gk_t = cst.tile([H, 1], F32, tag="gk")
            nc.sync.dma_start(gk_t[:], gk_d[:])
            ck_t = cst.tile([H, TKV], BF16, tag="ck")
            nc.sync.dma_start(ck_t[:], ck_d[:])
            sk_t = cst.tile([H, TKV], BF16, tag="sk")
            nc.sync.dma_start(sk_t[:], sk_d[:])
            mk_t = cst.tile([128, NQT * NWIN * 128], FP8, tag="mk")
            nc.sync.dma_start(
                mk_t[:].rearrange("p (q m) -> p q m", q=NQT),
                mk_d.rearrange("q p m -> p q m"))
            idb_t = cst.tile([128, 128], BF16, tag="idb")
            nc.sync.dma_start(idb_t[:], idb_d[:])
            ones_bf = cst.tile([128, 1], BF16, tag="ones")
            nc.vector.memset(ones_bf[:], 1.0)
            on1 = cst.tile([1, 128], F32, tag="on1")
            nc.vector.memset(on1[:], 1.0)
            id1 = cst.tile([1, 1], F32, tag="id1")
            nc.vector.memset(id1[:], 1.0)
            eps_t = cst.tile([1, 1], F32, tag="eps")
            nc.vector.memset(eps_t[:], EPS)


            # ---- big accumulators ----
            qTn = acc.tile([128, N * TQ], BF16, tag="qTn")
            kTn = acc.tile([128, K * TKV], BF16, tag="kTn")
            vsb = acc.tile([128, K * NST * VST], BF16, tag="vsb")
            nc.vector.memset(vsb[:], 1.0)
            encT = acc.tile([128, N * NQT * 128], BF16, tag="encT")
            # per-s-tile 1/(50*std_k) columns, [128 s, K*NST]
            rkc = acc.tile([128, K * NST], F32, tag="rkc")

            def rope(src_f32, c0, out_slice):
                rot = scr.tile([128, 512], F32, tag="rot")
                nc.vector.stream_shuffle(rot[:], src_f32[:], SWAP16)
                t1 = scr.tile([128, 512], F32, tag="t1")
                nc.vector.tensor_mul(t1[:], src_f32[:], ck_t[:, c0:c0 + 512])
                t2 = scr.tile([128, 512], F32, tag="t2")
                nc.vector.tensor_mul(t2[:], rot[:], sk_t[:, c0:c0 + 512])
                nc.vector.tensor_add(out_slice, t1[:], t2[:])

            def sumsq_rows(ps):
                """ps [128,512] f32 psum -> std [1,512] f32 = sqrt(ms+eps)."""
                sq = scr.tile([128, 512], BF16, tag="sq")
                nc.scalar.activation(sq[:], ps[:], AF.Square)
                ssp = psA.tile([1, 512], F32, tag="big")
                nc.tensor.matmul(ssp[:], ones_bf[:], sq[:],
                                 start=True, stop=True)
                std = scr.tile([1, 512], F32, tag="row")
                nc.scalar.activation(std[:], ssp[:], AF.Sqrt,
                                     scale=1.0 / H, bias=eps_t[:])
                return std

            # ---- phase 1: q heads (norm fully applied on q) ----
            q_s1 = q_s2 = None
            for n in range(N + 2):
                nstate = None
                if n < N:
                    if n in w_pre:
                        w_t = w_pre[n]
                    else:
                        w_t = wst.tile([128, ND * H], BF16, tag="w")
                        nc.sync.dma_start(
                            w_t[:].rearrange("p (d h) -> p d h", d=ND),
                            wq_d[n].rearrange("(d p) h -> p d h", p=128))
                    ps = psA.tile([128, 512], F32, tag="big")
                    for d in range(ND):
                        nc.tensor.matmul(
                            ps[:], w_t[:, d * H:(d + 1) * H],
                            xts[:, d * TKV + 1024:d * TKV + 1536],
                            start=(d == 0), stop=(d == ND - 1))
                    praw = scr.tile([128, 512], F32, tag="praw")
                    nc.vector.tensor_copy(praw[:], ps[:])
                    sq = scr.tile([128, 512], BF16, tag="sq")
                    nc.scalar.activation(sq[:], ps[:], AF.Square)
                    nstate = (n, praw, sq)
                if q_s1 is not None:
                    n1, praw1, sq1 = q_s1
                    ssp = psA.tile([1, 512], F32, tag="big")
                    nc.tensor.matmul(ssp[:], ones_bf[:], sq1[:],
                                     start=True, stop=True)
                    std = scr.tile([1, 512], F32, tag="row")
                    nc.scalar.activation(std[:], ssp[:], AF.Sqrt,
                                         scale=1.0 / H, bias=eps_t[:])
                    lnt = scr.tile([1, 512], F32, tag="row")
                    nc.scalar.activation(lnt[:], std[:], AF.Ln)
                    rst = scr.tile([1, 512], F32, tag="row")
                    nc.scalar.activation(rst[:], lnt[:], AF.Exp, scale=-1.0)
                    q_s1 = (n1, praw1, rst)
                if q_s2 is not None:
                    n2, praw2, rst2 = q_s2
                    rbp = psA.tile([128, 512], F32, tag="big")
                    nc.tensor.matmul(rbp[:], on1[:], rst2[:],
                                     start=True, stop=True)
                    qn = scr.tile([128, 512], F32, tag="qn")
                    nc.vector.scalar_tensor_tensor(
                        qn[:], praw2[:], gq_t[:], rbp[:],
                        op0=ALU.mult, op1=ALU.mult)
                    rope(qn, 1024, qTn[:, n2 * TQ:(n2 + 1) * TQ])
                q_s2 = q_s1
                q_s1 = nstate
            k_s1 = k_s2 = None
            rkps = {}
            nchunks = K * 3
            for ci in range(nchunks + 2):
                nstate = None
                if ci < nchunks:
                    kh, c = divmod(ci, 3)
                    if c == 0:
                        w_t = wst.tile([128, ND * H], BF16, tag="w")
                        nc.sync.dma_start(
                            w_t[:].rearrange("p (d h) -> p d h", d=ND),
                            wk_d[kh].rearrange("(d p) h -> p d h", p=128))
                        rkps[kh] = psB.tile([128, NST], F32, tag="sm",
                                            name=f"rkp_{kh}")
                    ps = psA.tile([128, 512], F32, tag="big")
                    for d in range(ND):
                        nc.tensor.matmul(
                            ps[:], w_t[:, d * H:(d + 1) * H],
                            xts[:, d * TKV + c * 512:d * TKV + (c + 1) * 512],
                            start=(d == 0), stop=(d == ND - 1))
                    kn = scr.tile([128, 512], F32, tag="kn")
                    nc.vector.tensor_scalar_mul(kn[:], ps[:], gk_t[:])
                    sq = scr.tile([128, 512], BF16, tag="sq")
                    nc.scalar.activation(sq[:], ps[:], AF.Square)
                    nstate = (kh, c, kn, sq)
                if k_s1 is not None:
                    kh1, c1, kn1, sq1 = k_s1
                    ssp = psA.tile([1, 512], F32, tag="big")
                    nc.tensor.matmul(ssp[:], ones_bf[:], sq1[:],
                                     start=True, stop=True)
                    std = scr.tile([1, 512], F32, tag="row")
                    nc.scalar.activation(std[:], ssp[:], AF.Sqrt,
                                         scale=1.0 / H, bias=eps_t[:])
                    k_s1 = (kh1, c1, kn1, std)
                if k_s2 is not None:
                    kh2, c2, kn2, std2 = k_s2
                    rkp2 = rkps[kh2]
                    for t4 in range(4):
                        st = c2 * 4 + t4
                        nc.tensor.matmul(
                            rkp2[:, st:st + 1],
                            std2[:, t4 * 128:(t4 + 1) * 128], id1[:],
                            is_transpose=True, start=True, stop=True)
                    rope(kn2, c2 * 512,
                         kTn[:, kh2 * TKV + c2 * 512:kh2 * TKV + (c2 + 1) * 512])
                    if c2 == 2:
                        rkraw = scr.tile([128, NST], F32, tag="rkraw")
                        nc.scalar.copy(rkraw[:], rkp2[:])
                        nc.vector.reciprocal(
                            rkc[:, kh2 * NST:(kh2 + 1) * NST], rkraw[:])
                        del rkps[kh2]
                k_s2 = k_s1
                k_s1 = nstate
            a_s1 = a_s2 = None
            iters = [(n, qi) for n in range(N) for qi in range(NQT)]
            a_it = [0]

            def attn_step(it):
                nstate = None
                if it < len(iters):
                        n, qi = iters[it]
                        kh = n // G
                        probs = pp.tile([128, NWIN * 128], BF16, tag="probs")
                        for c in range(3):
                            nr = 4 if c < 2 else 1
                            if c < 2:
                                lg = psA.tile([128, 512], F32, tag="big")
                            else:
                                lg = psB.tile([128, 128], F32, tag="sm")
                            for rr in range(nr):
                                r = c * 4 + rr
                                st = qi + r
                                nc.tensor.matmul(
                                    lg[:, rr * 128:(rr + 1) * 128],
                                    kTn[:, kh * TKV + st * 128:kh * TKV + (st + 1) * 128],
                                    qTn[:, n * TQ + qi * 128:n * TQ + (qi + 1) * 128],
                                    start=True, stop=True)
                            rk_sl = rkc[:, kh * NST + qi + c * 4:
                                        kh * NST + qi + c * 4 + nr]
                            rk_b = bass.AP(rk_sl.tensor, rk_sl.offset,
                                           list(rk_sl.ap) + [[0, 128]])
                            ttA = scr.tile([128, 512], F32, tag="ttA")
                            nc.vector.tensor_tensor(
                                ttA[:, :nr * 128].rearrange(
                                    "p (r t) -> p r t", r=nr),
                                lg[:].rearrange("p (r t) -> p r t", r=nr),
                                rk_b, op=ALU.mult)
                            ttB = scr.tile([128, 512], F32, tag="ttB")
                            nc.scalar.activation(ttB[:, :nr * 128],
                                                 ttA[:, :nr * 128],
                                                 AF.Tanh, scale=1.0 / SOFT_CAP)
                            ee = scr.tile([128, 512], BF16, tag="ee")
                            nc.scalar.activation(ee[:, :nr * 128],
                                                 ttB[:, :nr * 128],
                                                 AF.Exp, scale=SOFT_CAP)
                            nc.vector.tensor_mul(
                                probs[:, c * 512:c * 512 + nr * 128],
                                ee[:, :nr * 128],
                                mk_t[:, (qi * NWIN + c * 4) * 128:
                                     (qi * NWIN + c * 4 + nr) * 128])
                        nstate = (n, qi, probs)
                if a_s2 is not None:
                        n0, qi0, probs0 = a_s2
                        kh0 = n0 // G
                        ev = psB.tile([128, VST + 3], F32, tag="sm")
                        for r in range(NWIN):
                            st = qi0 + r
                            off = (kh0 * NST + st) * VST
                            nc.tensor.matmul(
                                ev[:, 0:VST],
                                probs0[:, r * 128:(r + 1) * 128],
                                vsb[:, off:off + VST],
                                start=(r == 0), stop=(r == NWIN - 1))
                        den = scr.tile([128, 1], F32, tag="den")
                        nc.vector.tensor_copy(den[:], ev[:, 128:129])
                        rden = scr.tile([128, 1], F32, tag="rden")
                        nc.vector.reciprocal(rden[:], den[:])
                        enc_sb = scr.tile([128, H], BF16, tag="encsb")
                        nc.vector.tensor_scalar_mul(enc_sb[:], ev[:, 0:H], rden[:])
                        etp = psB.tile([128, 128], BF16, tag="sm")
                        nc.tensor.matmul(etp[:], enc_sb[:], idb_t[:],
                                         is_transpose=True, start=True, stop=True)
                        nc.scalar.copy(
                            encT[:, (n0 * NQT + qi0) * 128:(n0 * NQT + qi0 + 1) * 128],
                            etp[:])
                return nstate

            def attn_advance(k_steps):
                nonlocal a_s1, a_s2
                for _ in range(k_steps):
                    if a_it[0] >= len(iters) + 2:
                        return
                    ns = attn_step(a_it[0])
                    a_it[0] += 1
                    a_s2 = a_s1
                    a_s1 = ns

            vstate = None
            for ci in range(nchunks + 1):
                nstate = None
                if ci < nchunks:
                    kh, c = divmod(ci, 3)
                    if c == 0:
                        w_t = wst.tile([128, ND * H], BF16, tag="w")
                        nc.sync.dma_start(
                            w_t[:].rearrange("p (d h) -> p d h", d=ND),
                            wv_d[kh].rearrange("(d p) h -> p d h", p=128))
                    ps = psA.tile([128, 512], F32, tag="big")
                    for d in range(ND):
                        nc.tensor.matmul(
                            ps[:], w_t[:, d * H:(d + 1) * H],
                            xts[:, d * TKV + c * 512:d * TKV + (c + 1) * 512],
                            start=(d == 0), stop=(d == ND - 1))
                    vt_sb = scr.tile([128, 512], BF16, tag="vt")
                    nc.vector.tensor_copy(vt_sb[:], ps[:])
                    nstate = (kh, c, vt_sb)
                if vstate is not None:
                    kh0, c0, vt0 = vstate
                    for t4 in range(4):
                        st = c0 * 4 + t4
                        tps = psB.tile([128, 128], BF16, tag="sm")
                        nc.tensor.matmul(
                            tps[:], vt0[:, t4 * 128:(t4 + 1) * 128],
                            idb_t[:], is_transpose=True,
                            start=True, stop=True)
                        off = (kh0 * NST + st) * VST
                        nc.scalar.copy(vsb[:, off:off + 128], tps[:])
                    if c0 == 2:
                        # v head kh0 complete: release its attention iters
                        attn_advance(2 * NQT)
                vstate = nstate
            attn_advance(len(iters) + 2 - a_it[0])

            # ---- phase 3: output projection ----
            for dc in range(4):
                ops = [psA.tile([128, 512], F32, tag="big", name=f"op_{dc}_{qi}")
                       for qi in range(NQT)]
                for n in range(N):
                    wo_sl = wost.tile([128, 512], BF16, tag="wo")
                    nc.sync.dma_start(wo_sl[:],
                                      wo_d[n][:, dc * 512:(dc + 1) * 512])
                    for qi in range(NQT):
                        nc.tensor.matmul(
                            ops[qi][:],
                            encT[:, (n * NQT + qi) * 128:(n * NQT + qi + 1) * 128],
                            wo_sl[:], start=(n == 0), stop=(n == N - 1))
                for qi in range(NQT):
                    osb = scr.tile([128, 512], F32, tag="osb")
                    nc.vector.tensor_copy(osb[:], ops[qi][:])
                    nc.sync.dma_start(
                        out_d[qi * 128:(qi + 1) * 128, dc * 512:(dc + 1) * 512],
                        osb[:])

    if split:
        _split_ctrl_multiwaits(nc)
    return nc


def _prep_inputs(x, q_w, kv_w, o_w, qnorm_scale, knorm_scale, segment_pos,
                 attn_mask):
    """Host-side shard + layout prep. Returns list of 8 input dicts."""
    bf = ml_dtypes.bfloat16
    f8 = ml_dtypes.float8_e4m3
    x = np.asarray(x, np.float32)
    q_w = np.asarray(q_w, np.float32)
    kv_w = np.asarray(kv_w, np.float32)
    o_w = np.asarray(o_w, np.float32)
    qnorm_scale = np.asarray(qnorm_scale, np.float32)
    knorm_scale = np.asarray(knorm_scale, np.float32)
    segment_pos = np.asarray(segment_pos, np.int64)
    attn_mask = np.asarray(attn_mask, bool)

    # shared (same array object across cores -> no copy)
    wq = np.ascontiguousarray(q_w[:, :, _ORIG]).astype(bf)
    wk = np.ascontiguousarray(kv_w[0][:, :, _ORIG]).astype(bf)
    wv = kv_w[1].astype(bf)
    wo = o_w.astype(bf)
    gq = ((1.0 + qnorm_scale[_ORIG]) * SCALE).reshape(H, 1).astype(np.float32)
    gk = (1.0 + knorm_scale[_ORIG]).reshape(H, 1).astype(np.float32)
    timescale = ROPE_BASE ** (2.0 * _FREQ.astype(np.float64) / H)  # [128]
    idb = np.eye(128, dtype=bf)

    in_maps = []
    for c in range(NCORES):
        b, j = divmod(c, NQT)
        qs = TQ * j
        kvs = qs - WINDOW

        # x^T for local kv window, zero-padded on the left
        xt = np.zeros((D, TKV), bf)
        lo = max(kvs, 0)
        xt[:, lo - kvs:] = x[b, lo:qs + TQ, :].T.astype(bf)

        # rope tables in permuted row order; positions from segment_pos
        pos = np.zeros(TKV, np.float64)
        pos[lo - kvs:] = segment_pos[b, lo:qs + TQ].astype(np.float64)
        theta = pos[None, :] / timescale[:, None]          # [128, TKV]
        ck = np.cos(theta).astype(bf)
        sk = (np.sin(theta) * _SIGN[:, None]).astype(bf)

        # masks [NQT, 128 s_p, NWIN*128 (r, t)] with validity baked in
        mk = np.zeros((NQT, 128, NWIN * 128), f8)
        seg = segment_pos[b]
        for qi in range(NQT):
            q_glob = qs + qi * 128 + np.arange(128)                  # [t]
            st = qi + np.arange(NWIN)
            k_glob = (kvs + st[:, None] * 128 + np.arange(128)[None, :])
            valid = k_glob >= 0                                       # [r, sp]
            k_safe = np.clip(k_glob, 0, T - 1)
            am = attn_mask[b][np.ix_(q_glob, k_safe.reshape(-1))]     # [t, r*sp]
            pk = seg[k_safe.reshape(-1)]                              # [r*sp]
            pq = seg[q_glob]                                          # [t]
            win = ((pk[None, :] > pq[:, None] - WINDOW)
                   & (pk[None, :] < pq[:, None] + WINDOW))
            m = (am & win & valid.reshape(1, -1)).astype(np.float32)  # [t, r*sp]
            m = m.reshape(128, NWIN, 128).transpose(2, 1, 0)          # [sp, r, t]
            mk[qi] = m.reshape(128, NWIN * 128).astype(f8)

        in_maps.append(dict(
            xt=xt, wq=wq, wk=wk, wv=wv, wo=wo, gq=gq, gk=gk,
            ck=np.ascontiguousarray(ck), sk=np.ascontiguousarray(sk),
            mk=mk, idb=idb))
    return in_maps


def kernel(x, q_w, kv_w, o_w, qnorm_scale, knorm_scale, segment_pos,
           attn_mask, _trace=False):
    import os
    if "nc" not in _module_cache:
        _module_cache["nc"] = _build_module()
    nc = _module_cache["nc"]

    in_maps = _prep_inputs(x, q_w, kv_w, o_w, qnorm_scale, knorm_scale,
                           segment_pos, attn_mask)
    res = run_bass_kernel_spmd(nc, in_maps, core_ids=list(range(NCORES)),
                               trace=_trace,
                               trace_cores=list(range(NCORES)) if _trace
                               else None)
    _module_cache["last_results"] = res

    out = np.zeros((B, T, D), np.float32)
    for c in range(NCORES):
        b, j = divmod(c, NQT)
        out[b, TQ * j:TQ * (j + 1), :] = res.results[c]["out"]
    return out
